# revision 41
# baseline (speedup 1.0000x reference)
"""Multi-head causal attention (B=2, S=2048, D=1024, H=16) on 8 trn2 cores.

Sharding: core c handles batch c//4 and heads 4*(c%4)..4*(c%4)+4 (256 channels).
Each core computes q/k/v projections for its channel slice, causal attention for
its 4 heads, and a partial output projection (contribution of its 256 channels
to the full [S, D] output). The host sums the 4 partials per batch and adds bo.

v2 schedule, built to keep the PE continuously busy (pstate ramp) and move all
softmax bookkeeping off the PE critical path:
  - qT/kT [256ch, 2048tok] via W-stationary matmuls over xT d-chunks; v in
    natural [tok, ch] layout with a ones column per head (v_aug) so the
    attention-value matmul also produces the softmax denominator; the ones
    columns are initialized by on-chip memset (no DRAM load).
  - scores^T for both heads of a 128-ch group land in one [128, 1024] PSUM
    tile (two banks); one exp covers both heads on off-diagonal k-tiles;
    diagonal k-tiles use per-head exp + DVE mask-multiply so the AV matmul
    never waits long. Masks are generated on-chip (gpsimd iota + compare).
  - scores for k-tile t+1 are emitted before the AV matmuls of k-tile t
    (depth-1 software pipeline) so the PE rarely waits on the exp chain.
  - normalization is fully off-PE: DVE reciprocal_approx_fast on the
    denominator row, gpsimd.partition_broadcast to spread it, DVE multiply.
  - input DMAs are issued in priority order (wk, xA-c0, wq, wv, ..., xA-c1)
    so the first projection chains start ~8us in; output stores ride the
    DVE queue, scratch copies the Sync queue.
"""

import sys

sys.path.insert(0, "/opt/trn_rl_repo")

import numpy as np
import concourse.bass as bass
import concourse.mybir as mybir
import concourse.tile as tile
from concourse.alu_op_type import AluOpType

F32R = mybir.dt.float32r
F32 = mybir.dt.float32
AF = mybir.ActivationFunctionType

D = 1024
S = 2048
B = 2
H = 16
DH = 64
CPC = 256  # channels per core (4 heads)
NKT = S // 128  # 16 k-tiles
V2W = 130  # [v0(64) | 1 | v1(64) | 1] per k-tile

_uid = [0]


def _split_waits(nc, max_waits=1):
    """This container's walrus rejects >max_waits sem-waits per instruction.
    Move excess waits onto preceding same-engine NoOps (one wait each);
    per-engine program order within a basic block preserves semantics."""
    n = 0
    for f in nc.m.functions:
        for b in f.blocks:
            insts = b.instructions
            if not any(
                i.sync_info is not None
                and i.sync_info.on_wait
                and len(i.sync_info.on_wait) > max_waits
                for i in insts
            ):
                continue
            new = []
            for inst in insts:
                si = inst.sync_info
                waits = list(si.on_wait) if si is not None and si.on_wait else []
                if len(waits) > max_waits:
                    for w in waits[max_waits:]:
                        _uid[0] += 1
                        new.append(
                            mybir.InstNoOp(
                                name=f"I-waitsplit-{_uid[0]}",
                                engine=inst.engine,
                                sync_info=mybir.SyncInfo(on_wait=[w], on_update=[]),
                            )
                        )
                        n += 1
                    si.on_wait = waits[:max_waits]
                new.append(inst)
            b.instructions = new
    return n


class _TC(tile.TileContext):
    def __exit__(self, exc_type, exc_val, exc_tb):
        r = super().__exit__(exc_type, exc_val, exc_tb)
        if exc_type is None:
            _split_waits(self.nc)
        return r


def _emit(nc, P, T):
    pc = P["const"]

    # --- constants / weights. wk/wq and the first x half are split lo/hi
    # (dc 0-3 / 4-7) so the first projection chains start ~4us in. ---
    wk_lo = pc.tile([128, 4 * CPC], F32R, tag="wk_lo", name="wk_lo")
    wk_hi = pc.tile([128, 4 * CPC], F32R, tag="wk_hi", name="wk_hi")
    wq_lo = pc.tile([128, 4 * CPC], F32R, tag="wq_lo", name="wq_lo")
    wq_hi = pc.tile([128, 4 * CPC], F32R, tag="wq_hi", name="wq_hi")
    wv_sb = pc.tile([128, 8 * CPC], F32R, tag="wv", name="wv")
    bq_sb = pc.tile([128, 2], F32, tag="bq", name="bq")
    bk_sb = pc.tile([128, 2], F32, tag="bk", name="bk")
    bvb = pc.tile([128, CPC], F32, tag="bvb", name="bvb")

    xa0_lo = P["xs"].tile([128, 4 * 512], F32R, tag="xs", name="xa0_lo")
    xa0_hi = P["xs"].tile([128, 4 * 512], F32R, tag="xs", name="xa0_hi")
    xa1 = P["x"].tile([128, 8 * 512], F32R, tag="x", name="xa1")

    # priority-ordered input loads on the Sync queue
    wdram = {"k": T["wk"], "q": T["wq"]}
    for w, half, dst in (("k", 0, wk_lo), ("k", 1, wk_hi)):
        nc.sync.dma_start(dst[:].rearrange("p (a c) -> p a c", a=4),
                          wdram[w].rearrange("(a p) c -> p a c", p=128)[:, 4 * half : 4 * half + 4])
    for half, dst in ((0, xa0_lo), (1, xa0_hi)):
        nc.sync.dma_start(dst[:].rearrange("p (a t) -> p a t", a=4),
                          T["xT"].rearrange("(a p) t -> p a t", p=128)[:, 4 * half : 4 * half + 4, 0:512])
    for w, half, dst in (("q", 0, wq_lo), ("q", 1, wq_hi)):
        nc.sync.dma_start(dst[:].rearrange("p (a c) -> p a c", a=4),
                          wdram[w].rearrange("(a p) c -> p a c", p=128)[:, 4 * half : 4 * half + 4])
    nc.sync.dma_start(wv_sb[:].rearrange("p (a c) -> p a c", a=8),
                      T["wv"].rearrange("(a p) c -> p a c", p=128))
    nc.sync.dma_start(bq_sb[:], T["bq"])
    nc.sync.dma_start(bk_sb[:], T["bk"])
    nc.sync.dma_start(bvb[:], T["bvf"])
    nc.sync.dma_start(xa1[:].rearrange("p (a t) -> p a t", a=8),
                      T["xT"].rearrange("(a p) t -> p a t", p=128)[:, :, 512:1024])

    wparts = {"k": (wk_lo, wk_hi), "q": (wq_lo, wq_hi)}
    xparts = {(0, 0): (xa0_lo, xa0_hi)}
    xh = {(0, 1): xa1}  # (half, c) -> whole tile
    wo_box = [None]

    def xap(half, ci, dc, o0, w):
        """AP for d-chunk dc, cols [o0:o0+w] of the (half, ci) x block."""
        if (half, ci) in xparts:
            t = xparts[(half, ci)][dc // 4]
            base = (dc % 4) * 512
        else:
            t = xh[(half, ci)]
            base = dc * 512
        return t[:, base + o0 : base + o0 + w]

    def wap(which, dc, g):
        t, base = (wparts[which][dc // 4], (dc % 4) * 256) if which in wparts else (None, 0)
        return t[:, base + g * 128 : base + g * 128 + 128]

    # causal masks [128 k, 2 heads x 512 q] per diagonal offset j, on-chip:
    # mask[j][k, :, q] = 1.0 if q - k - 128*j >= 0 else 0.0 (doubled so one
    # DVE multiply covers both heads of a merged exp tile)
    msk = []
    for j in range(4):
        m = pc.tile([128, 1024], F32R, tag=f"msk{j}", name=f"msk{j}")
        nc.gpsimd.iota(
            m[:],
            pattern=[[0, 2], [1, 512]],
            base=-128 * j,
            channel_multiplier=-1,
            allow_small_or_imprecise_dtypes=True,
        )
        nc.vector.tensor_scalar(m[:], m[:], 0.0, None, AluOpType.is_ge)
        msk.append(m)

    # f32r memset is broken in this walrus; build 1.0 columns via ACT
    # Identity with bias=1.0 from a zeroed f32 scratch (f32 memset works).
    zs = pc.tile([128, 64], F32, tag="zs", name="zs")
    nc.vector.memset(zs[:], 0.0)
    ones_sb = pc.tile([128, 64], F32R, tag="ones1", name="ones1")
    nc.scalar.activation(ones_sb[64:65, 0:64], zs[64:65, 0:64], AF.Identity, bias=1.0)

    # warm up the PE clock while the first loads are in flight: the pstate
    # ramp (0.65 -> 1.2 -> 2.4 GHz) needs continuous execution, so burn a few
    # dummy matmuls so real chains start closer to full speed
    wps = P["pa"].tile([128, 512], F32, tag="pa", name="warm")
    for _ in range(24):
        nc.tensor.matmul(
            wps[0:64, 0:64],
            ones_sb[64:65, 0:64],
            ones_sb[64:65, 0:64],
            start=True,
            stop=True,
        )

    # v_aug tiles; ones columns (64, 129) via the same ACT trick
    v2 = [
        [
            P["v2"].tile([128, V2W], F32R, tag=f"v2_{g}_{tt}", name=f"v2_{g}_{tt}")
            for tt in range(NKT)
        ]
        for g in range(2)
    ]
    for g in range(2):
        for tt in range(NKT):
            nc.scalar.activation(
                v2[g][tt][:].rearrange("p (a c) -> p a c", a=2)[:, :, 64:65],
                zs[:, 0:2],
                AF.Identity,
                bias=1.0,
            )

    qt = {}  # (g, c) -> [128ch, 512tok]
    kt = {
        (g, c): P["kt"].tile([128, 512], F32R, tag=f"kt{g}_{c}", name=f"kt{g}_{c}")
        for g in range(2)
        for c in range(4)
    }
    ot = {}  # (g, qc) -> [128ch, 512tok] normalized attention output

    def qk_chain(which, g, c):
        """One q or k projection chain: 8 accumulating matmuls + bias-evac."""
        half, ci = divmod(c, 2)
        bsb = bk_sb if which == "k" else bq_sb
        ps = P["pa"].tile([128, 512], F32, tag="pa", name="pa")
        for dc in range(8):
            yield "pe", lambda ps=ps, dc=dc, which=which, half=half, ci=ci, g=g: nc.tensor.matmul(
                ps[:],
                wap(which, dc, g),
                xap(half, ci, dc, 0, 512),
                start=(dc == 0),
                stop=(dc == 7),
            )

        def _evac(ps=ps, which=which, g=g, c=c, bsb=bsb):
            if which == "k":
                dst = kt[(g, c)]
            else:
                dst = P["qt"].tile([128, 512], F32R, tag="qt", name=f"qt{g}_{c}")
                qt[(g, c)] = dst
            nc.vector.tensor_scalar_add(dst[:], ps[:], bsb[:, g : g + 1])

        yield "x", _evac

    def v_chain(tt):
        """One v projection chain (128 tokens, all 256 ch): 8 matmuls + 2 evacs."""
        half, tl = divmod(tt, 8)
        ci, t128 = divmod(tl, 4)
        ps = P["pa"].tile([128, 512], F32, tag="pa", name="pa")
        for dc in range(8):
            yield "pe", lambda ps=ps, dc=dc, half=half, ci=ci, t128=t128: nc.tensor.matmul(
                ps[:, 0:256],
                xap(half, ci, dc, t128 * 128, 128),
                wv_sb[:, dc * 256 : (dc + 1) * 256],
                start=(dc == 0),
                stop=(dc == 7),
            )
        for g in range(2):
            yield "x", lambda ps=ps, g=g, tt=tt: nc.vector.tensor_add(
                v2[g][tt][:].rearrange("p (a c) -> p a c", a=2)[:, :, 0:64],
                ps[:, g * 128 : (g + 1) * 128].rearrange("p (a c) -> p a c", a=2),
                bvb[:, g * 128 : (g + 1) * 128].rearrange("p (a c) -> p a c", a=2),
            )

    def outproj(qc):
        """Partial out-projection for 512 tokens: 16 matmuls + evac/store."""
        for tl in range(4):
            tt = qc * 4 + tl
            for nch in range(2):
                ps = P["pa"].tile([128, 512], F32, tag="pa", name="pa")
                for g in range(2):
                    yield "pe", lambda ps=ps, g=g, qc=qc, tl=tl, nch=nch: nc.tensor.matmul(
                        ps[:],
                        ot[(g, qc)][:, tl * 128 : (tl + 1) * 128],
                        wo_box[0][:, g * D + nch * 512 : g * D + nch * 512 + 512],
                        start=(g == 0),
                        stop=(g == 1),
                    )

                def _evac(ps=ps, tt=tt, nch=nch):
                    ob = P["ob"].tile([128, 512], F32, tag="ob", name="ob")
                    # alternate evac engine so neither DVE nor ACT serializes
                    # the pa-pool rotation in the drain tail
                    if nch == 0:
                        nc.vector.tensor_scalar_add(ob[:], ps[:], 0.0)
                    else:
                        nc.scalar.copy(ob[:], ps[:])
                    nc.sync.dma_start(
                        T["out"][tt * 128 : (tt + 1) * 128, nch * 512 : (nch + 1) * 512],
                        ob[:],
                    )

                yield "x", _evac

    # --- filler machinery: inject deferred PE work between attention matmuls.
    # Units are ("pe", fn) or ("x", fn); fill(n) emits until n PE units ran so
    # evac/recip units don't eat the PE's fill budget. ---
    fill_q = []
    norm_gens = set()

    def fill(n_pe, cap=8):
        while n_pe > 0 and cap > 0 and fill_q:
            u = next(fill_q[0], None)
            if u is None:
                norm_gens.discard(fill_q[0])
                fill_q.pop(0)
                continue
            kind, fn = u
            fn()
            cap -= 1
            if kind == "pe":
                n_pe -= 1

    def drain_front():
        gen = fill_q.pop(0)
        norm_gens.discard(gen)
        for _, fn in gen:
            fn()

    def drain_until(marker):
        """Drain everything queued ahead of (and including) marker, but keep
        norm generators paced via fill slots — bursting them would park the
        PE behind the ACT reciprocal chain."""
        stash = []
        while any(g is marker for g in fill_q):
            if fill_q[0] in norm_gens:
                stash.append(fill_q.pop(0))
                continue
            drain_front()
        fill_q[0:0] = stash

    def attention_seg(qc, g):
        """Causal attention for one 128-ch head pair over 512 queries.

        AV matmuls trail the score matmuls by two k-tiles so the exp+mask
        chain (ACT+DVE) never gates the PE; sc double-buffering gives the
        same slack to the score matmuls."""
        Oh = [None, None]
        nkt = 4 * qc + 4
        es = {}

        def av_pair(kti):
            if Oh[0] is None:
                Oh[0] = P["po"].tile([128, 512], F32, tag="po", name="po")
                Oh[1] = P["po"].tile([128, 512], F32, tag="po", name="po")
            e, off = es.pop(kti)
            for par in range(2):
                nc.tensor.matmul(
                    Oh[par][0:65, off:512],
                    v2[g][kti][:, par * 65 : par * 65 + 65],
                    e[:, par * 512 + off : (par + 1) * 512],
                    start=(kti == 0),
                    stop=(kti == nkt - 1),
                )

        for kti in range(nkt):
            ktile = kt[(g, kti // 4)]
            k0 = (kti % 4) * 128
            j = kti - 4 * qc
            # diag tiles: only cols [off:] are unmasked; keep N>=256 for f32r
            off = 0 if j < 1 else min(128 * j, 256)
            sc = P["sc"].tile([128, 1024], F32, tag="sc", name="sc")
            for par in range(2):
                nc.tensor.matmul(
                    sc[:, par * 512 + off : (par + 1) * 512],
                    ktile[64 * par : 64 * par + 64, k0 : k0 + 128],
                    qt[(g, qc)][64 * par : 64 * par + 64, off:512],
                    start=True,
                    stop=True,
                )
            e = P["e"].tile([128, 1024], F32R, tag="e", name="e")
            ev = e[:].rearrange("p (a q) -> p a q", a=2)[:, :, off:512]
            scv = sc[:].rearrange("p (a q) -> p a q", a=2)[:, :, off:512]
            nc.scalar.activation(ev, scv, AF.Exp)
            if j >= 0:
                mv = msk[j][:].rearrange("p (a q) -> p a q", a=2)[:, :, off:512]
                nc.vector.tensor_mul(ev, ev, mv)
            es[kti] = (e, off)
            fill(2)
            if kti >= 2:
                av_pair(kti - 2)
        av_pair(nkt - 2)
        fill(2)
        av_pair(nkt - 1)

        # evacuate the unnormalized head outputs (+ denominator row) to SBUF
        # right away so the two PSUM banks recycle for the next segment
        uo = []
        for par in range(2):
            u = P["uo"].tile([128, 512], F32R, tag="uo", name="uo")
            nc.vector.tensor_scalar_add(u[0:65, :], Oh[par][0:65, :], 0.0)
            uo.append(u)

        # normalization, fully deferred through the fill queue so the PE and
        # ACT streams never stall on it: ACT rl2 = exp(-ln(l)) = 1/l on the
        # denominator row, then PE ones-matmul broadcast, DVE evac + multiply.
        dst = P["ot"].tile([128, 512], F32R, tag="ot", name=f"ot{g}_{qc}")
        ot[(g, qc)] = dst

        def norm_units(uo=uo, dst=dst):
            rl2s = []

            def _recip(par):
                rl = P["rl"].tile([128, 512], F32R, tag="rl", name="rl")
                with nc.allow_low_precision(reason="softmax denominators, ~1e-3"):
                    nc.scalar.activation(rl[64:65, :], uo[par][64:65, :], AF.Ln)
                    rl2 = P["rl2"].tile([128, 512], F32R, tag="rl2", name="rl2")
                    nc.scalar.activation(
                        rl2[64:65, :], rl[64:65, :], AF.Exp, scale=-1.0
                    )
                rl2s.append(rl2)

            yield "x", lambda: _recip(0)
            yield "x", lambda: _recip(1)
            for par in range(2):
                rlb = P["pa"].tile([128, 512], F32, tag="pa", name="rlb")
                yield "pe", lambda rlb=rlb, par=par: nc.tensor.matmul(
                    rlb[0:64, :],
                    ones_sb[64:65, 0:64],
                    rl2s[par][64:65, :],
                    start=True,
                    stop=True,
                )

                def _mul(rlb=rlb, par=par):
                    rlb_sb = P["rl"].tile([128, 512], F32R, tag="rlbsb", name="rlbsb")
                    nc.vector.tensor_scalar_add(rlb_sb[0:64, :], rlb[0:64, :], 0.0)
                    if par == 0:
                        nc.vector.tensor_mul(
                            dst[0:64, :], uo[par][0:64, :], rlb_sb[0:64, :]
                        )
                    else:
                        tmp = P["rl"].tile([128, 512], F32R, tag="otmp", name="otmp")
                        nc.vector.tensor_mul(
                            tmp[0:64, :], uo[par][0:64, :], rlb_sb[0:64, :]
                        )
                        nc.sync.dma_start(dst[64:128, :], tmp[0:64, :])

                yield "x", _mul

        gen = norm_units()
        norm_gens.add(gen)
        fill_q.insert(0, gen)

    # ---- schedule ----
    # inline: the chains the first attention segment needs
    for which, g in (("k", 0), ("k", 1), ("q", 0), ("q", 1)):
        for _, fn in qk_chain(which, g, 0):
            fn()
    for tt in range(4):
        for _, fn in v_chain(tt):
            fn()

    def xbwo_loads():
        """Half-B x loads + wo. Deferred until all xa1 readers are emitted so
        xb1 can recycle xa1's pool buffer (x pool bufs=2)."""
        def _go():
            xb0 = P["x"].tile([128, 8 * 512], F32R, tag="x", name="xb0")
            xb1 = P["x"].tile([128, 8 * 512], F32R, tag="x", name="xb1")
            nc.sync.dma_start(xb0[:].rearrange("p (a t) -> p a t", a=8),
                              T["xT"].rearrange("(a p) t -> p a t", p=128)[:, :, 1024:1536])
            nc.sync.dma_start(xb1[:].rearrange("p (a t) -> p a t", a=8),
                              T["xT"].rearrange("(a p) t -> p a t", p=128)[:, :, 1536:2048])
            wo = pc.tile([128, 2 * D], F32R, tag="wo", name="wo")
            nc.sync.dma_start(wo[:].rearrange("p (t n) -> p t n", t=2),
                              T["wo"].rearrange("(t p) n -> p t n", p=128))
            xh[(1, 0)] = xb0
            xh[(1, 1)] = xb1
            wo_box[0] = wo

        yield "x", _go

    # deferred work, in the order later segments need it
    for which, g in (("k", 0), ("k", 1), ("q", 0), ("q", 1)):
        fill_q.append(qk_chain(which, g, 1))
    pre_qc = {}
    for tt in range(4, 8):
        gen = v_chain(tt)
        fill_q.append(gen)
    pre_qc[1] = gen
    fill_q.append(xbwo_loads())
    for which, g in (("k", 0), ("k", 1), ("q", 0), ("q", 1)):
        fill_q.append(qk_chain(which, g, 2))
    for tt in range(8, 12):
        gen = v_chain(tt)
        fill_q.append(gen)
    pre_qc[2] = gen
    for which, g in (("k", 0), ("k", 1), ("q", 0), ("q", 1)):
        fill_q.append(qk_chain(which, g, 3))
    for tt in range(12, 16):
        gen = v_chain(tt)
        fill_q.append(gen)
    pre_qc[3] = gen

    for qc in range(4):
        if qc in pre_qc:
            drain_until(pre_qc[qc])
        for g in range(2):
            attention_seg(qc, g)
        fill_q.append(outproj(qc))
    # final drain: give the last norm's ACT reciprocal time to land by
    # emitting one other generator's PE work ahead of it. The last element is
    # always outproj(3), which needs the norm emitted first — only swap when
    # fill_q[1] is an earlier (already-satisfied) generator.
    if fill_q and fill_q[0] in norm_gens and len(fill_q) > 2:
        fill_q[0], fill_q[1] = fill_q[1], fill_q[0]
    while fill_q:
        drain_front()


def build(reps=1, with_bias=True, hw_loop=0):
    nc = bass.Bass("TRN2", target_bir_lowering=False, debug=False, num_devices=8)
    T = {
        "xT": nc.dram_tensor("xT", [D, S], F32R, kind="ExternalInput").ap(),
        "wq": nc.dram_tensor("wq", [D, CPC], F32R, kind="ExternalInput").ap(),
        "wk": nc.dram_tensor("wk", [D, CPC], F32R, kind="ExternalInput").ap(),
        "wv": nc.dram_tensor("wv", [D, CPC], F32R, kind="ExternalInput").ap(),
        "wo": nc.dram_tensor("wo", [CPC, D], F32R, kind="ExternalInput").ap(),
        "bq": nc.dram_tensor("bq", [128, 2], F32, kind="ExternalInput").ap(),
        "bk": nc.dram_tensor("bk", [128, 2], F32, kind="ExternalInput").ap(),
        "bvf": nc.dram_tensor("bvf", [128, CPC], F32, kind="ExternalInput").ap(),
        "out": nc.dram_tensor("out", [S, D], F32, kind="ExternalOutput").ap(),
    }
    with _TC(nc) as tc:
        with (
            tc.tile_pool(name="const", bufs=1) as p_const,
            tc.tile_pool(name="x", bufs=2) as p_x,
            tc.tile_pool(name="xs", bufs=2) as p_xs,
            tc.tile_pool(name="qt", bufs=4) as p_qt,
            tc.tile_pool(name="kt", bufs=1) as p_kt,
            tc.tile_pool(name="v2", bufs=1) as p_v2,
            tc.tile_pool(name="e", bufs=3) as p_e,
            tc.tile_pool(name="ot", bufs=8) as p_ot,
            tc.tile_pool(name="uo", bufs=4) as p_uo,
            tc.tile_pool(name="rl", bufs=1) as p_rl,
            tc.tile_pool(name="rl2", bufs=2) as p_rl2,
            tc.tile_pool(name="ob", bufs=2) as p_ob,
            tc.tile_pool(name="pa", bufs=2, space="PSUM") as p_pa,
            tc.tile_pool(name="sc", bufs=2, space="PSUM") as p_sc,
            tc.tile_pool(name="po", bufs=2, space="PSUM") as p_po,
        ):
            P = {
                "const": p_const,
                "x": p_x,
                "xs": p_xs,
                "qt": p_qt,
                "kt": p_kt,
                "v2": p_v2,
                "e": p_e,
                "ot": p_ot,
                "uo": p_uo,
                "rl": p_rl,
                "rl2": p_rl2,
                "ob": p_ob,
                "pa": p_pa,
                "sc": p_sc,
                "po": p_po,
            }
            if hw_loop:
                with tc.For_i(0, hw_loop, 1):
                    _emit(nc, P, T)
            else:
                for _ in range(reps):
                    _emit(nc, P, T)
    return nc


def make_in_maps(x, Wq, bq, Wk, bk, Wv, bv, Wo, bo):
    """Host-side sharding: returns per-core input dicts."""
    scale = 1.0 / np.sqrt(np.float32(DH))
    xTs = [np.ascontiguousarray(x[b].T) for b in range(B)]
    in_maps = []
    for c in range(8):
        b = c // 4
        t = c % 4
        ch0 = t * CPC
        in_maps.append(
            {
                "xT": xTs[b],
                "wq": np.ascontiguousarray(Wq[:, ch0 : ch0 + CPC]) * scale,
                "wk": np.ascontiguousarray(Wk[:, ch0 : ch0 + CPC]),
                "wv": np.ascontiguousarray(Wv[:, ch0 : ch0 + CPC]),
                "wo": np.ascontiguousarray(Wo[ch0 : ch0 + CPC, :]),
                "bq": np.ascontiguousarray(
                    (bq[ch0 : ch0 + CPC] * scale).reshape(2, 128).T
                ),
                "bk": np.ascontiguousarray(bk[ch0 : ch0 + CPC].reshape(2, 128).T),
                "bvf": np.ascontiguousarray(np.broadcast_to(bv[ch0 : ch0 + CPC], (128, CPC))),
            }
        )
    return in_maps


def combine(results, bo):
    """Sum the 4 per-batch partials and add bo -> [B, S, D]."""
    out = np.zeros((B, S, D), np.float32)
    for c in range(8):
        out[c // 4] += results[c]["out"]
    return (out + bo.reshape(1, 1, D)).astype(np.float32)


def kernel(x, Wq, bq, Wk, bk, Wv, bv, Wo, bo):
    from concourse.bass_utils import run_bass_kernel_spmd

    args = [np.asarray(a, np.float32) for a in (x, Wq, bq, Wk, bk, Wv, bv, Wo, bo)]
    x, Wq, bq, Wk, bk, Wv, bv, Wo, bo = args
    nc = build(reps=1)
    in_maps = make_in_maps(x, Wq, bq, Wk, bk, Wv, bv, Wo, bo)
    res = run_bass_kernel_spmd(nc, in_maps, core_ids=list(range(8)))
    return combine(res.results, bo)


# revision 47
# speedup vs baseline: 1.0443x; 1.0443x over previous
"""Multi-head causal attention (B=2, S=2048, D=1024, H=16) on 8 trn2 cores.

Sharding: core c handles batch c//4 and heads 4*(c%4)..4*(c%4)+4 (256 channels).
Each core computes q/k/v projections for its channel slice, causal attention for
its 4 heads, and a partial output projection (contribution of its 256 channels
to the full [S, D] output). The host sums the 4 partials per batch and adds bo.

v2 schedule, built to keep the PE continuously busy (pstate ramp) and move all
softmax bookkeeping off the PE critical path:
  - qT/kT [256ch, 2048tok] via W-stationary matmuls over xT d-chunks; v in
    natural [tok, ch] layout with a ones column per head (v_aug) so the
    attention-value matmul also produces the softmax denominator; the ones
    columns are initialized by on-chip memset (no DRAM load).
  - scores^T for both heads of a 128-ch group land in one [128, 1024] PSUM
    tile (two banks); one exp covers both heads on off-diagonal k-tiles;
    diagonal k-tiles use per-head exp + DVE mask-multiply so the AV matmul
    never waits long. Masks are generated on-chip (gpsimd iota + compare).
  - scores for k-tile t+1 are emitted before the AV matmuls of k-tile t
    (depth-1 software pipeline) so the PE rarely waits on the exp chain.
  - normalization is fully off-PE: DVE reciprocal_approx_fast on the
    denominator row, gpsimd.partition_broadcast to spread it, DVE multiply.
  - input DMAs are issued in priority order (wk, xA-c0, wq, wv, ..., xA-c1)
    so the first projection chains start ~8us in; output stores ride the
    DVE queue, scratch copies the Sync queue.
"""

import sys

sys.path.insert(0, "/opt/trn_rl_repo")

import numpy as np
import concourse.bass as bass
import concourse.mybir as mybir
import concourse.tile as tile
from concourse.alu_op_type import AluOpType

F32R = mybir.dt.float32r
F32 = mybir.dt.float32
AF = mybir.ActivationFunctionType

D = 1024
S = 2048
B = 2
H = 16
DH = 64
CPC = 256  # channels per core (4 heads)
NKT = S // 128  # 16 k-tiles
V2W = 130  # [v0(64) | 1 | v1(64) | 1] per k-tile

_uid = [0]


def _split_waits(nc, max_waits=1):
    """This container's walrus rejects >max_waits sem-waits per instruction.
    Move excess waits onto preceding same-engine NoOps (one wait each);
    per-engine program order within a basic block preserves semantics."""
    n = 0
    for f in nc.m.functions:
        for b in f.blocks:
            insts = b.instructions
            if not any(
                i.sync_info is not None
                and i.sync_info.on_wait
                and len(i.sync_info.on_wait) > max_waits
                for i in insts
            ):
                continue
            new = []
            for inst in insts:
                si = inst.sync_info
                waits = list(si.on_wait) if si is not None and si.on_wait else []
                if len(waits) > max_waits:
                    for w in waits[max_waits:]:
                        _uid[0] += 1
                        new.append(
                            mybir.InstNoOp(
                                name=f"I-waitsplit-{_uid[0]}",
                                engine=inst.engine,
                                sync_info=mybir.SyncInfo(on_wait=[w], on_update=[]),
                            )
                        )
                        n += 1
                    si.on_wait = waits[:max_waits]
                new.append(inst)
            b.instructions = new
    return n


class _TC(tile.TileContext):
    def __exit__(self, exc_type, exc_val, exc_tb):
        r = super().__exit__(exc_type, exc_val, exc_tb)
        if exc_type is None:
            _split_waits(self.nc)
        return r


def _emit(nc, P, T):
    pc = P["const"]

    # --- constants / weights (mega tiles: [128 d-rows, 8 dc-chunks x cols]) ---
    wk_sb = pc.tile([128, 8 * CPC], F32R, tag="wk", name="wk")
    wq_sb = pc.tile([128, 8 * CPC], F32R, tag="wq", name="wq")
    wv_sb = pc.tile([128, 8 * CPC], F32R, tag="wv", name="wv")
    bq_sb = pc.tile([128, 2], F32, tag="bq", name="bq")
    bk_sb = pc.tile([128, 2], F32, tag="bk", name="bk")
    bvb = pc.tile([128, CPC], F32, tag="bvb", name="bvb")

    xa0 = P["x"].tile([128, 8 * 512], F32R, tag="x", name="xa0")
    xa1 = P["x"].tile([128, 8 * 512], F32R, tag="x", name="xa1")

    # priority-ordered input loads on the Sync queue
    nc.sync.dma_start(wk_sb[:].rearrange("p (a c) -> p a c", a=8),
                      T["wk"].rearrange("(a p) c -> p a c", p=128))
    nc.sync.dma_start(xa0[:].rearrange("p (a t) -> p a t", a=8),
                      T["xT"].rearrange("(a p) t -> p a t", p=128)[:, :, 0:512])
    nc.sync.dma_start(wq_sb[:].rearrange("p (a c) -> p a c", a=8),
                      T["wq"].rearrange("(a p) c -> p a c", p=128))
    nc.sync.dma_start(wv_sb[:].rearrange("p (a c) -> p a c", a=8),
                      T["wv"].rearrange("(a p) c -> p a c", p=128))
    nc.sync.dma_start(bq_sb[:], T["bq"])
    nc.sync.dma_start(bk_sb[:], T["bk"])
    nc.sync.dma_start(bvb[:], T["bvf"])
    nc.sync.dma_start(xa1[:].rearrange("p (a t) -> p a t", a=8),
                      T["xT"].rearrange("(a p) t -> p a t", p=128)[:, :, 512:1024])

    wparts = {"k": wk_sb, "q": wq_sb}
    xh = {(0, 0): xa0, (0, 1): xa1}  # (half, c) -> tile
    wo_box = [None]

    def xap(half, ci, dc, o0, w):
        """AP for d-chunk dc, cols [o0:o0+w] of the (half, ci) x block."""
        t = xh[(half, ci)]
        base = dc * 512
        return t[:, base + o0 : base + o0 + w]

    def wap(which, dc, g):
        t = wparts[which]
        base = dc * 256
        return t[:, base + g * 128 : base + g * 128 + 128]

    # causal masks [128 k, 2 heads x 512 q] per diagonal offset j, on-chip:
    # mask[j][k, :, q] = 1.0 if q - k - 128*j >= 0 else 0.0 (doubled so one
    # DVE multiply covers both heads of a merged exp tile)
    msk = []
    for j in range(4):
        m = pc.tile([128, 1024], F32R, tag=f"msk{j}", name=f"msk{j}")
        nc.gpsimd.iota(
            m[:],
            pattern=[[0, 2], [1, 512]],
            base=-128 * j,
            channel_multiplier=-1,
            allow_small_or_imprecise_dtypes=True,
        )
        nc.vector.tensor_scalar(m[:], m[:], 0.0, None, AluOpType.is_ge)
        msk.append(m)

    # f32r memset is broken in this walrus; build 1.0 columns via ACT
    # Identity with bias=1.0 from a zeroed f32 scratch (f32 memset works).
    zs = pc.tile([128, 64], F32, tag="zs", name="zs")
    nc.vector.memset(zs[:], 0.0)
    ones_sb = pc.tile([128, 64], F32R, tag="ones1", name="ones1")
    nc.scalar.activation(ones_sb[64:65, 0:64], zs[64:65, 0:64], AF.Identity, bias=1.0)



    # v_aug tiles; ones columns (64, 129) via the same ACT trick
    v2 = [
        [
            P["v2"].tile([128, V2W], F32R, tag=f"v2_{g}_{tt}", name=f"v2_{g}_{tt}")
            for tt in range(NKT)
        ]
        for g in range(2)
    ]
    for g in range(2):
        for tt in range(NKT):
            nc.scalar.activation(
                v2[g][tt][:].rearrange("p (a c) -> p a c", a=2)[:, :, 64:65],
                zs[:, 0:2],
                AF.Identity,
                bias=1.0,
            )

    qt = {}  # (g, c) -> [128ch, 512tok]
    kt = {
        (g, c): P["kt"].tile([128, 512], F32R, tag=f"kt{g}_{c}", name=f"kt{g}_{c}")
        for g in range(2)
        for c in range(4)
    }
    ot = {}  # (g, qc) -> [128ch, 512tok] normalized attention output

    def qk_chain(which, g, c):
        """One q or k projection chain: 8 accumulating matmuls + bias-evac."""
        half, ci = divmod(c, 2)
        bsb = bk_sb if which == "k" else bq_sb
        ps = P["pa"].tile([128, 512], F32, tag="pa", name="pa")
        for dc in range(8):
            yield "pe", lambda ps=ps, dc=dc, which=which, half=half, ci=ci, g=g: nc.tensor.matmul(
                ps[:],
                wap(which, dc, g),
                xap(half, ci, dc, 0, 512),
                start=(dc == 0),
                stop=(dc == 7),
            )

        def _evac(ps=ps, which=which, g=g, c=c, bsb=bsb):
            if which == "k":
                dst = kt[(g, c)]
            else:
                dst = P["qt"].tile([128, 512], F32R, tag="qt", name=f"qt{g}_{c}")
                qt[(g, c)] = dst
            nc.vector.tensor_scalar_add(dst[:], ps[:], bsb[:, g : g + 1])

        yield "x", _evac

    def v_chain(tt):
        """One v projection chain (128 tokens, all 256 ch): 8 matmuls + 2 evacs."""
        half, tl = divmod(tt, 8)
        ci, t128 = divmod(tl, 4)
        ps = P["pa"].tile([128, 512], F32, tag="pa", name="pa")
        for dc in range(8):
            yield "pe", lambda ps=ps, dc=dc, half=half, ci=ci, t128=t128: nc.tensor.matmul(
                ps[:, 0:256],
                xap(half, ci, dc, t128 * 128, 128),
                wv_sb[:, dc * 256 : (dc + 1) * 256],
                start=(dc == 0),
                stop=(dc == 7),
            )
        for g in range(2):
            yield "x", lambda ps=ps, g=g, tt=tt: nc.vector.tensor_add(
                v2[g][tt][:].rearrange("p (a c) -> p a c", a=2)[:, :, 0:64],
                ps[:, g * 128 : (g + 1) * 128].rearrange("p (a c) -> p a c", a=2),
                bvb[:, g * 128 : (g + 1) * 128].rearrange("p (a c) -> p a c", a=2),
            )

    def outproj(qc):
        """Partial out-projection for 512 tokens: 16 matmuls + evac/store."""
        for tl in range(4):
            tt = qc * 4 + tl
            for nch in range(2):
                ps = P["pa"].tile([128, 512], F32, tag="pa", name="pa")
                for g in range(2):
                    yield "pe", lambda ps=ps, g=g, qc=qc, tl=tl, nch=nch: nc.tensor.matmul(
                        ps[:],
                        ot[(g, qc)][:, tl * 128 : (tl + 1) * 128],
                        wo_box[0][:, g * D + nch * 512 : g * D + nch * 512 + 512],
                        start=(g == 0),
                        stop=(g == 1),
                    )

                def _evac(ps=ps, tt=tt, nch=nch):
                    ob = P["ob"].tile([128, 512], F32, tag="ob", name="ob")
                    # alternate evac engine so neither DVE nor ACT serializes
                    # the pa-pool rotation in the drain tail
                    if nch == 0:
                        nc.vector.tensor_scalar_add(ob[:], ps[:], 0.0)
                    else:
                        nc.scalar.copy(ob[:], ps[:])
                    nc.sync.dma_start(
                        T["out"][tt * 128 : (tt + 1) * 128, nch * 512 : (nch + 1) * 512],
                        ob[:],
                    )

                yield "x", _evac

    # --- filler machinery: inject deferred PE work between attention matmuls.
    # Units are ("pe", fn) or ("x", fn); fill(n) emits until n PE units ran so
    # evac/recip units don't eat the PE's fill budget. ---
    fill_q = []
    norm_gens = set()

    def fill(n_pe, cap=8):
        while n_pe > 0 and cap > 0 and fill_q:
            u = next(fill_q[0], None)
            if u is None:
                norm_gens.discard(fill_q[0])
                fill_q.pop(0)
                continue
            kind, fn = u
            fn()
            cap -= 1
            if kind == "pe":
                n_pe -= 1

    def drain_front():
        gen = fill_q.pop(0)
        norm_gens.discard(gen)
        for _, fn in gen:
            fn()

    def drain_until(marker):
        """Drain everything queued ahead of (and including) marker, but keep
        norm generators paced via fill slots — bursting them would park the
        PE behind the ACT reciprocal chain."""
        stash = []
        while any(g is marker for g in fill_q):
            if fill_q[0] in norm_gens:
                stash.append(fill_q.pop(0))
                continue
            drain_front()
        fill_q[0:0] = stash

    def attention_seg(qc, g):
        """Causal attention for one 128-ch head pair over 512 queries.

        AV matmuls trail the score matmuls by two k-tiles so the exp+mask
        chain (ACT+DVE) never gates the PE; sc double-buffering gives the
        same slack to the score matmuls."""
        Oh = [None, None]
        nkt = 4 * qc + 4
        es = {}

        def av_pair(kti):
            if Oh[0] is None:
                Oh[0] = P["po"].tile([128, 512], F32, tag="po", name="po")
                Oh[1] = P["po"].tile([128, 512], F32, tag="po", name="po")
            e, off = es.pop(kti)
            for par in range(2):
                nc.tensor.matmul(
                    Oh[par][0:65, off:512],
                    v2[g][kti][:, par * 65 : par * 65 + 65],
                    e[:, par * 512 + off : (par + 1) * 512],
                    start=(kti == 0),
                    stop=(kti == nkt - 1),
                )

        for kti in range(nkt):
            ktile = kt[(g, kti // 4)]
            k0 = (kti % 4) * 128
            j = kti - 4 * qc
            # diag tiles: only cols [off:] are unmasked; keep N>=256 for f32r
            off = 0 if j < 1 else min(128 * j, 256)
            sc = P["sc"].tile([128, 1024], F32, tag="sc", name="sc")
            for par in range(2):
                nc.tensor.matmul(
                    sc[:, par * 512 + off : (par + 1) * 512],
                    ktile[64 * par : 64 * par + 64, k0 : k0 + 128],
                    qt[(g, qc)][64 * par : 64 * par + 64, off:512],
                    start=True,
                    stop=True,
                )
            e = P["e"].tile([128, 1024], F32R, tag="e", name="e")
            ev = e[:].rearrange("p (a q) -> p a q", a=2)[:, :, off:512]
            scv = sc[:].rearrange("p (a q) -> p a q", a=2)[:, :, off:512]
            nc.scalar.activation(ev, scv, AF.Exp)
            if j >= 0:
                mv = msk[j][:].rearrange("p (a q) -> p a q", a=2)[:, :, off:512]
                nc.vector.tensor_mul(ev, ev, mv)
            es[kti] = (e, off)
            fill(2)
            if kti >= 2:
                av_pair(kti - 2)
        av_pair(nkt - 2)
        fill(2)
        av_pair(nkt - 1)

        # evacuate the unnormalized head outputs (+ denominator row) to SBUF
        # right away so the two PSUM banks recycle for the next segment
        uo = []
        for par in range(2):
            u = P["uo"].tile([128, 512], F32R, tag="uo", name="uo")
            nc.vector.tensor_scalar_add(u[0:65, :], Oh[par][0:65, :], 0.0)
            uo.append(u)

        # normalization, fully deferred through the fill queue so the PE and
        # ACT streams never stall on it: ACT rl2 = exp(-ln(l)) = 1/l on the
        # denominator row, then PE ones-matmul broadcast, DVE evac + multiply.
        dst = P["ot"].tile([128, 512], F32R, tag="ot", name=f"ot{g}_{qc}")
        ot[(g, qc)] = dst

        def norm_units(uo=uo, dst=dst):
            rl2s = []

            def _recip(par):
                rl = P["rl"].tile([128, 512], F32R, tag="rl", name="rl")
                with nc.allow_low_precision(reason="softmax denominators, ~1e-3"):
                    nc.scalar.activation(rl[64:65, :], uo[par][64:65, :], AF.Ln)
                    rl2 = P["rl2"].tile([128, 512], F32R, tag="rl2", name="rl2")
                    nc.scalar.activation(
                        rl2[64:65, :], rl[64:65, :], AF.Exp, scale=-1.0
                    )
                rl2s.append(rl2)

            yield "x", lambda: _recip(0)
            yield "x", lambda: _recip(1)
            for par in range(2):
                rlb = P["pa"].tile([128, 512], F32, tag="pa", name="rlb")
                yield "pe", lambda rlb=rlb, par=par: nc.tensor.matmul(
                    rlb[0:64, :],
                    ones_sb[64:65, 0:64],
                    rl2s[par][64:65, :],
                    start=True,
                    stop=True,
                )

                def _mul(rlb=rlb, par=par):
                    rlb_sb = P["rl"].tile([128, 512], F32R, tag="rlbsb", name="rlbsb")
                    nc.vector.tensor_scalar_add(rlb_sb[0:64, :], rlb[0:64, :], 0.0)
                    if par == 0:
                        nc.vector.tensor_mul(
                            dst[0:64, :], uo[par][0:64, :], rlb_sb[0:64, :]
                        )
                    else:
                        tmp = P["rl"].tile([128, 512], F32R, tag="otmp", name="otmp")
                        nc.vector.tensor_mul(
                            tmp[0:64, :], uo[par][0:64, :], rlb_sb[0:64, :]
                        )
                        nc.sync.dma_start(dst[64:128, :], tmp[0:64, :])

                yield "x", _mul

        gen = norm_units()
        norm_gens.add(gen)
        fill_q.insert(0, gen)

    # ---- schedule ----
    # inline: the chains the first attention segment needs
    for which, g in (("k", 0), ("k", 1), ("q", 0), ("q", 1)):
        for _, fn in qk_chain(which, g, 0):
            fn()
    for tt in range(4):
        for _, fn in v_chain(tt):
            fn()

    # half-B x loads + wo: xb0 reuses xa0's pool buffer, whose readers (the
    # c0 chains above) are all emitted; triggers queue behind the startup
    # loads on the Sync FIFO, preserving bandwidth priority.
    xb0 = P["x"].tile([128, 8 * 512], F32R, tag="x", name="xb0")
    xb1 = P["x"].tile([128, 8 * 512], F32R, tag="x", name="xb1")
    nc.sync.dma_start(xb0[:].rearrange("p (a t) -> p a t", a=8),
                      T["xT"].rearrange("(a p) t -> p a t", p=128)[:, :, 1024:1536])
    nc.sync.dma_start(xb1[:].rearrange("p (a t) -> p a t", a=8),
                      T["xT"].rearrange("(a p) t -> p a t", p=128)[:, :, 1536:2048])
    wo = pc.tile([128, 2 * D], F32R, tag="wo", name="wo")
    nc.sync.dma_start(wo[:].rearrange("p (t n) -> p t n", t=2),
                      T["wo"].rearrange("(t p) n -> p t n", p=128))
    xh[(1, 0)] = xb0
    xh[(1, 1)] = xb1
    wo_box[0] = wo

    # deferred work, in the order later segments need it
    for which, g in (("k", 0), ("k", 1), ("q", 0), ("q", 1)):
        fill_q.append(qk_chain(which, g, 1))
    pre_qc = {}
    for tt in range(4, 8):
        gen = v_chain(tt)
        fill_q.append(gen)
    pre_qc[1] = gen
    for which, g in (("k", 0), ("k", 1), ("q", 0), ("q", 1)):
        fill_q.append(qk_chain(which, g, 2))
    for tt in range(8, 12):
        gen = v_chain(tt)
        fill_q.append(gen)
    pre_qc[2] = gen
    for which, g in (("k", 0), ("k", 1), ("q", 0), ("q", 1)):
        fill_q.append(qk_chain(which, g, 3))
    for tt in range(12, 16):
        gen = v_chain(tt)
        fill_q.append(gen)
    pre_qc[3] = gen

    for qc in range(4):
        if qc in pre_qc:
            drain_until(pre_qc[qc])
        for g in range(2):
            attention_seg(qc, g)
        fill_q.append(outproj(qc))
    # final drain: give the last norm's ACT reciprocal time to land by
    # emitting one other generator's PE work ahead of it. The last element is
    # always outproj(3), which needs the norm emitted first — only swap when
    # fill_q[1] is an earlier (already-satisfied) generator.
    if fill_q and fill_q[0] in norm_gens and len(fill_q) > 2:
        fill_q[0], fill_q[1] = fill_q[1], fill_q[0]
    while fill_q:
        drain_front()


def build(reps=1, with_bias=True, hw_loop=0):
    nc = bass.Bass("TRN2", target_bir_lowering=False, debug=False, num_devices=8)
    T = {
        "xT": nc.dram_tensor("xT", [D, S], F32R, kind="ExternalInput").ap(),
        "wq": nc.dram_tensor("wq", [D, CPC], F32R, kind="ExternalInput").ap(),
        "wk": nc.dram_tensor("wk", [D, CPC], F32R, kind="ExternalInput").ap(),
        "wv": nc.dram_tensor("wv", [D, CPC], F32R, kind="ExternalInput").ap(),
        "wo": nc.dram_tensor("wo", [CPC, D], F32R, kind="ExternalInput").ap(),
        "bq": nc.dram_tensor("bq", [128, 2], F32, kind="ExternalInput").ap(),
        "bk": nc.dram_tensor("bk", [128, 2], F32, kind="ExternalInput").ap(),
        "bvf": nc.dram_tensor("bvf", [128, CPC], F32, kind="ExternalInput").ap(),
        "out": nc.dram_tensor("out", [S, D], F32, kind="ExternalOutput").ap(),
    }
    with _TC(nc) as tc:
        with (
            tc.tile_pool(name="const", bufs=1) as p_const,
            tc.tile_pool(name="x", bufs=3) as p_x,
            tc.tile_pool(name="qt", bufs=4) as p_qt,
            tc.tile_pool(name="kt", bufs=1) as p_kt,
            tc.tile_pool(name="v2", bufs=1) as p_v2,
            tc.tile_pool(name="e", bufs=3) as p_e,
            tc.tile_pool(name="ot", bufs=8) as p_ot,
            tc.tile_pool(name="uo", bufs=4) as p_uo,
            tc.tile_pool(name="rl", bufs=2) as p_rl,
            tc.tile_pool(name="rl2", bufs=2) as p_rl2,
            tc.tile_pool(name="ob", bufs=3) as p_ob,
            tc.tile_pool(name="pa", bufs=2, space="PSUM") as p_pa,
            tc.tile_pool(name="sc", bufs=2, space="PSUM") as p_sc,
            tc.tile_pool(name="po", bufs=2, space="PSUM") as p_po,
        ):
            P = {
                "const": p_const,
                "x": p_x,
                "qt": p_qt,
                "kt": p_kt,
                "v2": p_v2,
                "e": p_e,
                "ot": p_ot,
                "uo": p_uo,
                "rl": p_rl,
                "rl2": p_rl2,
                "ob": p_ob,
                "pa": p_pa,
                "sc": p_sc,
                "po": p_po,
            }
            if hw_loop:
                with tc.For_i(0, hw_loop, 1):
                    _emit(nc, P, T)
            else:
                for _ in range(reps):
                    _emit(nc, P, T)
    return nc


def make_in_maps(x, Wq, bq, Wk, bk, Wv, bv, Wo, bo):
    """Host-side sharding: returns per-core input dicts."""
    scale = 1.0 / np.sqrt(np.float32(DH))
    xTs = [np.ascontiguousarray(x[b].T) for b in range(B)]
    in_maps = []
    for c in range(8):
        b = c // 4
        t = c % 4
        ch0 = t * CPC
        in_maps.append(
            {
                "xT": xTs[b],
                "wq": np.ascontiguousarray(Wq[:, ch0 : ch0 + CPC]) * scale,
                "wk": np.ascontiguousarray(Wk[:, ch0 : ch0 + CPC]),
                "wv": np.ascontiguousarray(Wv[:, ch0 : ch0 + CPC]),
                "wo": np.ascontiguousarray(Wo[ch0 : ch0 + CPC, :]),
                "bq": np.ascontiguousarray(
                    (bq[ch0 : ch0 + CPC] * scale).reshape(2, 128).T
                ),
                "bk": np.ascontiguousarray(bk[ch0 : ch0 + CPC].reshape(2, 128).T),
                "bvf": np.ascontiguousarray(np.broadcast_to(bv[ch0 : ch0 + CPC], (128, CPC))),
            }
        )
    return in_maps


def combine(results, bo):
    """Sum the 4 per-batch partials and add bo -> [B, S, D]."""
    out = np.zeros((B, S, D), np.float32)
    for c in range(8):
        out[c // 4] += results[c]["out"]
    return (out + bo.reshape(1, 1, D)).astype(np.float32)


def kernel(x, Wq, bq, Wk, bk, Wv, bv, Wo, bo):
    from concourse.bass_utils import run_bass_kernel_spmd

    args = [np.asarray(a, np.float32) for a in (x, Wq, bq, Wk, bk, Wv, bv, Wo, bo)]
    x, Wq, bq, Wk, bk, Wv, bv, Wo, bo = args
    nc = build(reps=1)
    in_maps = make_in_maps(x, Wq, bq, Wk, bk, Wv, bv, Wo, bo)
    res = run_bass_kernel_spmd(nc, in_maps, core_ids=list(range(8)))
    return combine(res.results, bo)


# revision 48
# speedup vs baseline: 1.0539x; 1.0092x over previous
"""Multi-head causal attention (B=2, S=2048, D=1024, H=16) on 8 trn2 cores.

Sharding: core c handles batch c//4 and heads 4*(c%4)..4*(c%4)+4 (256 channels).
Each core computes q/k/v projections for its channel slice, causal attention for
its 4 heads, and a partial output projection (contribution of its 256 channels
to the full [S, D] output). The host sums the 4 partials per batch and adds bo.

v2 schedule, built to keep the PE continuously busy (pstate ramp) and move all
softmax bookkeeping off the PE critical path:
  - qT/kT [256ch, 2048tok] via W-stationary matmuls over xT d-chunks; v in
    natural [tok, ch] layout with a ones column per head (v_aug) so the
    attention-value matmul also produces the softmax denominator; the ones
    columns are initialized by on-chip memset (no DRAM load).
  - scores^T for both heads of a 128-ch group land in one [128, 1024] PSUM
    tile (two banks); one exp covers both heads on off-diagonal k-tiles;
    diagonal k-tiles use per-head exp + DVE mask-multiply so the AV matmul
    never waits long. Masks are generated on-chip (gpsimd iota + compare).
  - scores for k-tile t+1 are emitted before the AV matmuls of k-tile t
    (depth-1 software pipeline) so the PE rarely waits on the exp chain.
  - normalization is fully off-PE: DVE reciprocal_approx_fast on the
    denominator row, gpsimd.partition_broadcast to spread it, DVE multiply.
  - input DMAs are issued in priority order (wk, xA-c0, wq, wv, ..., xA-c1)
    so the first projection chains start ~8us in; output stores ride the
    DVE queue, scratch copies the Sync queue.
"""

import sys

sys.path.insert(0, "/opt/trn_rl_repo")

import numpy as np
import concourse.bass as bass
import concourse.mybir as mybir
import concourse.tile as tile
from concourse.alu_op_type import AluOpType

F32R = mybir.dt.float32r
F32 = mybir.dt.float32
AF = mybir.ActivationFunctionType

D = 1024
S = 2048
B = 2
H = 16
DH = 64
CPC = 256  # channels per core (4 heads)
NKT = S // 128  # 16 k-tiles
V2W = 130  # [v0(64) | 1 | v1(64) | 1] per k-tile

_uid = [0]


def _split_waits(nc, max_waits=1):
    """This container's walrus rejects >max_waits sem-waits per instruction.
    Move excess waits onto preceding same-engine NoOps (one wait each);
    per-engine program order within a basic block preserves semantics."""
    n = 0
    for f in nc.m.functions:
        for b in f.blocks:
            insts = b.instructions
            if not any(
                i.sync_info is not None
                and i.sync_info.on_wait
                and len(i.sync_info.on_wait) > max_waits
                for i in insts
            ):
                continue
            new = []
            for inst in insts:
                si = inst.sync_info
                waits = list(si.on_wait) if si is not None and si.on_wait else []
                if len(waits) > max_waits:
                    for w in waits[max_waits:]:
                        _uid[0] += 1
                        new.append(
                            mybir.InstNoOp(
                                name=f"I-waitsplit-{_uid[0]}",
                                engine=inst.engine,
                                sync_info=mybir.SyncInfo(on_wait=[w], on_update=[]),
                            )
                        )
                        n += 1
                    si.on_wait = waits[:max_waits]
                new.append(inst)
            b.instructions = new
    return n


class _TC(tile.TileContext):
    def __exit__(self, exc_type, exc_val, exc_tb):
        r = super().__exit__(exc_type, exc_val, exc_tb)
        if exc_type is None:
            _split_waits(self.nc)
        return r


def _emit(nc, P, T):
    pc = P["const"]

    # --- constants / weights (mega tiles: [128 d-rows, 8 dc-chunks x cols]) ---
    wk_sb = pc.tile([128, 8 * CPC], F32R, tag="wk", name="wk")
    wq_sb = pc.tile([128, 8 * CPC], F32R, tag="wq", name="wq")
    wv_sb = pc.tile([128, 8 * CPC], F32R, tag="wv", name="wv")
    bq_sb = pc.tile([128, 2], F32, tag="bq", name="bq")
    bk_sb = pc.tile([128, 2], F32, tag="bk", name="bk")
    bvb = pc.tile([128, CPC], F32, tag="bvb", name="bvb")

    xa0 = P["x"].tile([128, 8 * 512], F32R, tag="x", name="xa0")
    xa1 = P["x"].tile([128, 8 * 512], F32R, tag="x", name="xa1")

    # priority-ordered input loads on the Sync queue
    nc.sync.dma_start(wk_sb[:].rearrange("p (a c) -> p a c", a=8),
                      T["wk"].rearrange("(a p) c -> p a c", p=128))
    nc.sync.dma_start(xa0[:].rearrange("p (a t) -> p a t", a=8),
                      T["xT"].rearrange("(a p) t -> p a t", p=128)[:, :, 0:512])
    nc.sync.dma_start(wq_sb[:].rearrange("p (a c) -> p a c", a=8),
                      T["wq"].rearrange("(a p) c -> p a c", p=128))
    nc.sync.dma_start(wv_sb[:].rearrange("p (a c) -> p a c", a=8),
                      T["wv"].rearrange("(a p) c -> p a c", p=128))
    nc.sync.dma_start(bq_sb[:], T["bq"])
    nc.sync.dma_start(bk_sb[:], T["bk"])
    nc.sync.dma_start(bvb[:], T["bvf"])
    nc.sync.dma_start(xa1[:].rearrange("p (a t) -> p a t", a=8),
                      T["xT"].rearrange("(a p) t -> p a t", p=128)[:, :, 512:1024])

    wparts = {"k": wk_sb, "q": wq_sb}
    xh = {(0, 0): xa0, (0, 1): xa1}  # (half, c) -> tile
    wo_box = [None]

    def xap(half, ci, dc, o0, w):
        """AP for d-chunk dc, cols [o0:o0+w] of the (half, ci) x block."""
        t = xh[(half, ci)]
        base = dc * 512
        return t[:, base + o0 : base + o0 + w]

    def wap(which, dc, g):
        t = wparts[which]
        base = dc * 256
        return t[:, base + g * 128 : base + g * 128 + 128]

    # causal masks [128 k, 2 heads x 512 q] per diagonal offset j, on-chip:
    # mask[j][k, :, q] = 1.0 if q - k - 128*j >= 0 else 0.0 (doubled so one
    # DVE multiply covers both heads of a merged exp tile)
    msk = []
    for j in range(4):
        m = pc.tile([128, 1024], F32R, tag=f"msk{j}", name=f"msk{j}")
        nc.gpsimd.iota(
            m[:],
            pattern=[[0, 2], [1, 512]],
            base=-128 * j,
            channel_multiplier=-1,
            allow_small_or_imprecise_dtypes=True,
        )
        nc.vector.tensor_scalar(m[:], m[:], 0.0, None, AluOpType.is_ge)
        msk.append(m)

    # f32r memset is broken in this walrus; build 1.0 columns via ACT
    # Identity with bias=1.0 from a zeroed f32 scratch (f32 memset works).
    zs = pc.tile([128, 64], F32, tag="zs", name="zs")
    nc.vector.memset(zs[:], 0.0)
    ones_sb = pc.tile([128, 64], F32R, tag="ones1", name="ones1")
    nc.scalar.activation(ones_sb[64:65, 0:64], zs[64:65, 0:64], AF.Identity, bias=1.0)



    # v_aug tiles; ones columns (64, 129) via the same ACT trick
    v2 = [
        [
            P["v2"].tile([128, V2W], F32R, tag=f"v2_{g}_{tt}", name=f"v2_{g}_{tt}")
            for tt in range(NKT)
        ]
        for g in range(2)
    ]
    for g in range(2):
        for tt in range(NKT):
            nc.scalar.activation(
                v2[g][tt][:].rearrange("p (a c) -> p a c", a=2)[:, :, 64:65],
                zs[:, 0:2],
                AF.Identity,
                bias=1.0,
            )

    qt = {}  # (g, c) -> [128ch, 512tok]
    kt = {
        (g, c): P["kt"].tile([128, 512], F32R, tag=f"kt{g}_{c}", name=f"kt{g}_{c}")
        for g in range(2)
        for c in range(4)
    }
    ot = {}  # (g, qc) -> [128ch, 512tok] normalized attention output

    def qk_chain(which, g, c):
        """One q or k projection chain: 8 accumulating matmuls + bias-evac."""
        half, ci = divmod(c, 2)
        bsb = bk_sb if which == "k" else bq_sb
        ps = P["pa"].tile([128, 512], F32, tag="pa", name="pa")
        for dc in range(8):
            yield "pe", lambda ps=ps, dc=dc, which=which, half=half, ci=ci, g=g: nc.tensor.matmul(
                ps[:],
                wap(which, dc, g),
                xap(half, ci, dc, 0, 512),
                start=(dc == 0),
                stop=(dc == 7),
            )

        def _evac(ps=ps, which=which, g=g, c=c, bsb=bsb):
            if which == "k":
                dst = kt[(g, c)]
            else:
                dst = P["qt"].tile([128, 512], F32R, tag="qt", name=f"qt{g}_{c}")
                qt[(g, c)] = dst
            nc.vector.tensor_scalar_add(dst[:], ps[:], bsb[:, g : g + 1])

        yield "x", _evac

    def v_chain(tt):
        """One v projection chain (128 tokens, all 256 ch): 8 matmuls + 2 evacs."""
        half, tl = divmod(tt, 8)
        ci, t128 = divmod(tl, 4)
        ps = P["pa"].tile([128, 512], F32, tag="pa", name="pa")
        for dc in range(8):
            yield "pe", lambda ps=ps, dc=dc, half=half, ci=ci, t128=t128: nc.tensor.matmul(
                ps[:, 0:256],
                xap(half, ci, dc, t128 * 128, 128),
                wv_sb[:, dc * 256 : (dc + 1) * 256],
                start=(dc == 0),
                stop=(dc == 7),
            )
        for g in range(2):
            yield "x", lambda ps=ps, g=g, tt=tt: nc.vector.tensor_add(
                v2[g][tt][:].rearrange("p (a c) -> p a c", a=2)[:, :, 0:64],
                ps[:, g * 128 : (g + 1) * 128].rearrange("p (a c) -> p a c", a=2),
                bvb[:, g * 128 : (g + 1) * 128].rearrange("p (a c) -> p a c", a=2),
            )

    def outproj(qc):
        """Partial out-projection for 512 tokens: 16 matmuls + evac/store."""
        for tl in range(4):
            tt = qc * 4 + tl
            for nch in range(2):
                ps = P["pa"].tile([128, 512], F32, tag="pa", name="pa")
                for g in range(2):
                    yield "pe", lambda ps=ps, g=g, qc=qc, tl=tl, nch=nch: nc.tensor.matmul(
                        ps[:],
                        ot[(g, qc)][:, tl * 128 : (tl + 1) * 128],
                        wo_box[0][:, g * D + nch * 512 : g * D + nch * 512 + 512],
                        start=(g == 0),
                        stop=(g == 1),
                    )

                def _evac(ps=ps, tt=tt, nch=nch):
                    ob = P["ob"].tile([128, 512], F32, tag="ob", name="ob")
                    # alternate evac engine so neither DVE nor ACT serializes
                    # the pa-pool rotation in the drain tail
                    if nch == 0:
                        nc.vector.tensor_scalar_add(ob[:], ps[:], 0.0)
                    else:
                        nc.scalar.copy(ob[:], ps[:])
                    nc.sync.dma_start(
                        T["out"][tt * 128 : (tt + 1) * 128, nch * 512 : (nch + 1) * 512],
                        ob[:],
                    )

                yield "x", _evac

    # --- filler machinery: inject deferred PE work between attention matmuls.
    # Units are ("pe", fn) or ("x", fn); fill(n) emits until n PE units ran so
    # evac/recip units don't eat the PE's fill budget. ---
    fill_q = []
    norm_gens = set()

    def fill(n_pe, cap=8):
        while n_pe > 0 and cap > 0 and fill_q:
            u = next(fill_q[0], None)
            if u is None:
                norm_gens.discard(fill_q[0])
                fill_q.pop(0)
                continue
            kind, fn = u
            fn()
            cap -= 1
            if kind == "pe":
                n_pe -= 1

    def drain_front():
        gen = fill_q.pop(0)
        norm_gens.discard(gen)
        for _, fn in gen:
            fn()

    def drain_until(marker):
        """Drain everything queued ahead of (and including) marker, but keep
        norm generators paced via fill slots — bursting them would park the
        PE behind the ACT reciprocal chain."""
        stash = []
        while any(g is marker for g in fill_q):
            if fill_q[0] in norm_gens:
                stash.append(fill_q.pop(0))
                continue
            drain_front()
        fill_q[0:0] = stash

    def attention_seg(qc, g):
        """Causal attention for one 128-ch head pair over 512 queries.

        AV matmuls trail the score matmuls by two k-tiles so the exp+mask
        chain (ACT+DVE) never gates the PE; sc double-buffering gives the
        same slack to the score matmuls."""
        Oh = [None, None]
        nkt = 4 * qc + 4
        es = {}

        def av_pair(kti):
            if Oh[0] is None:
                Oh[0] = P["po"].tile([128, 512], F32, tag="po", name="po")
                Oh[1] = P["po"].tile([128, 512], F32, tag="po", name="po")
            e, off = es.pop(kti)
            for par in range(2):
                nc.tensor.matmul(
                    Oh[par][0:65, off:512],
                    v2[g][kti][:, par * 65 : par * 65 + 65],
                    e[:, par * 512 + off : (par + 1) * 512],
                    start=(kti == 0),
                    stop=(kti == nkt - 1),
                )

        for kti in range(nkt):
            ktile = kt[(g, kti // 4)]
            k0 = (kti % 4) * 128
            j = kti - 4 * qc
            # diag tiles: only cols [off:] are unmasked; keep N>=256 for f32r
            off = 0 if j < 1 else min(128 * j, 256)
            sc = P["sc"].tile([128, 1024], F32, tag="sc", name="sc")
            for par in range(2):
                # explicit row-tiles: the two K=64 score matmuls run
                # concurrently in disjoint halves of the PE array
                nc.tensor.matmul(
                    sc[:, par * 512 + off : (par + 1) * 512],
                    ktile[64 * par : 64 * par + 64, k0 : k0 + 128],
                    qt[(g, qc)][64 * par : 64 * par + 64, off:512],
                    start=True,
                    stop=True,
                    tile_position=(64 * par, 0),
                )
            e = P["e"].tile([128, 1024], F32R, tag="e", name="e")
            ev = e[:].rearrange("p (a q) -> p a q", a=2)[:, :, off:512]
            scv = sc[:].rearrange("p (a q) -> p a q", a=2)[:, :, off:512]
            nc.scalar.activation(ev, scv, AF.Exp)
            if j >= 0:
                mv = msk[j][:].rearrange("p (a q) -> p a q", a=2)[:, :, off:512]
                nc.vector.tensor_mul(ev, ev, mv)
            es[kti] = (e, off)
            fill(2)
            if kti >= 2:
                av_pair(kti - 2)
        av_pair(nkt - 2)
        fill(2)
        av_pair(nkt - 1)

        # evacuate the unnormalized head outputs (+ denominator row) to SBUF
        # right away so the two PSUM banks recycle for the next segment
        uo = []
        for par in range(2):
            u = P["uo"].tile([128, 512], F32R, tag="uo", name="uo")
            nc.vector.tensor_scalar_add(u[0:65, :], Oh[par][0:65, :], 0.0)
            uo.append(u)

        # normalization, fully deferred through the fill queue so the PE and
        # ACT streams never stall on it: ACT rl2 = exp(-ln(l)) = 1/l on the
        # denominator row, then PE ones-matmul broadcast, DVE evac + multiply.
        dst = P["ot"].tile([128, 512], F32R, tag="ot", name=f"ot{g}_{qc}")
        ot[(g, qc)] = dst

        def norm_units(uo=uo, dst=dst):
            rl2s = []

            def _recip(par):
                rl = P["rl"].tile([128, 512], F32R, tag="rl", name="rl")
                with nc.allow_low_precision(reason="softmax denominators, ~1e-3"):
                    nc.scalar.activation(rl[64:65, :], uo[par][64:65, :], AF.Ln)
                    rl2 = P["rl2"].tile([128, 512], F32R, tag="rl2", name="rl2")
                    nc.scalar.activation(
                        rl2[64:65, :], rl[64:65, :], AF.Exp, scale=-1.0
                    )
                rl2s.append(rl2)

            yield "x", lambda: _recip(0)
            yield "x", lambda: _recip(1)
            for par in range(2):
                rlb = P["pa"].tile([128, 512], F32, tag="pa", name="rlb")
                yield "pe", lambda rlb=rlb, par=par: nc.tensor.matmul(
                    rlb[0:64, :],
                    ones_sb[64:65, 0:64],
                    rl2s[par][64:65, :],
                    start=True,
                    stop=True,
                )

                def _mul(rlb=rlb, par=par):
                    rlb_sb = P["rl"].tile([128, 512], F32R, tag="rlbsb", name="rlbsb")
                    nc.vector.tensor_scalar_add(rlb_sb[0:64, :], rlb[0:64, :], 0.0)
                    if par == 0:
                        nc.vector.tensor_mul(
                            dst[0:64, :], uo[par][0:64, :], rlb_sb[0:64, :]
                        )
                    else:
                        tmp = P["rl"].tile([128, 512], F32R, tag="otmp", name="otmp")
                        nc.vector.tensor_mul(
                            tmp[0:64, :], uo[par][0:64, :], rlb_sb[0:64, :]
                        )
                        nc.sync.dma_start(dst[64:128, :], tmp[0:64, :])

                yield "x", _mul

        gen = norm_units()
        norm_gens.add(gen)
        fill_q.insert(0, gen)

    # ---- schedule ----
    # inline: the chains the first attention segment needs
    for which, g in (("k", 0), ("k", 1), ("q", 0), ("q", 1)):
        for _, fn in qk_chain(which, g, 0):
            fn()
    for tt in range(4):
        for _, fn in v_chain(tt):
            fn()

    # half-B x loads + wo: xb0 reuses xa0's pool buffer, whose readers (the
    # c0 chains above) are all emitted; triggers queue behind the startup
    # loads on the Sync FIFO, preserving bandwidth priority.
    xb0 = P["x"].tile([128, 8 * 512], F32R, tag="x", name="xb0")
    xb1 = P["x"].tile([128, 8 * 512], F32R, tag="x", name="xb1")
    nc.sync.dma_start(xb0[:].rearrange("p (a t) -> p a t", a=8),
                      T["xT"].rearrange("(a p) t -> p a t", p=128)[:, :, 1024:1536])
    nc.sync.dma_start(xb1[:].rearrange("p (a t) -> p a t", a=8),
                      T["xT"].rearrange("(a p) t -> p a t", p=128)[:, :, 1536:2048])
    wo = pc.tile([128, 2 * D], F32R, tag="wo", name="wo")
    nc.sync.dma_start(wo[:].rearrange("p (t n) -> p t n", t=2),
                      T["wo"].rearrange("(t p) n -> p t n", p=128))
    xh[(1, 0)] = xb0
    xh[(1, 1)] = xb1
    wo_box[0] = wo

    # deferred work, in the order later segments need it
    for which, g in (("k", 0), ("k", 1), ("q", 0), ("q", 1)):
        fill_q.append(qk_chain(which, g, 1))
    pre_qc = {}
    for tt in range(4, 8):
        gen = v_chain(tt)
        fill_q.append(gen)
    pre_qc[1] = gen
    for which, g in (("k", 0), ("k", 1), ("q", 0), ("q", 1)):
        fill_q.append(qk_chain(which, g, 2))
    for tt in range(8, 12):
        gen = v_chain(tt)
        fill_q.append(gen)
    pre_qc[2] = gen
    for which, g in (("k", 0), ("k", 1), ("q", 0), ("q", 1)):
        fill_q.append(qk_chain(which, g, 3))
    for tt in range(12, 16):
        gen = v_chain(tt)
        fill_q.append(gen)
    pre_qc[3] = gen

    for qc in range(4):
        if qc in pre_qc:
            drain_until(pre_qc[qc])
        for g in range(2):
            attention_seg(qc, g)
        fill_q.append(outproj(qc))
    # final drain: give the last norm's ACT reciprocal time to land by
    # emitting one other generator's PE work ahead of it. The last element is
    # always outproj(3), which needs the norm emitted first — only swap when
    # fill_q[1] is an earlier (already-satisfied) generator.
    if fill_q and fill_q[0] in norm_gens and len(fill_q) > 2:
        fill_q[0], fill_q[1] = fill_q[1], fill_q[0]
    while fill_q:
        drain_front()


def build(reps=1, with_bias=True, hw_loop=0):
    nc = bass.Bass("TRN2", target_bir_lowering=False, debug=False, num_devices=8)
    T = {
        "xT": nc.dram_tensor("xT", [D, S], F32R, kind="ExternalInput").ap(),
        "wq": nc.dram_tensor("wq", [D, CPC], F32R, kind="ExternalInput").ap(),
        "wk": nc.dram_tensor("wk", [D, CPC], F32R, kind="ExternalInput").ap(),
        "wv": nc.dram_tensor("wv", [D, CPC], F32R, kind="ExternalInput").ap(),
        "wo": nc.dram_tensor("wo", [CPC, D], F32R, kind="ExternalInput").ap(),
        "bq": nc.dram_tensor("bq", [128, 2], F32, kind="ExternalInput").ap(),
        "bk": nc.dram_tensor("bk", [128, 2], F32, kind="ExternalInput").ap(),
        "bvf": nc.dram_tensor("bvf", [128, CPC], F32, kind="ExternalInput").ap(),
        "out": nc.dram_tensor("out", [S, D], F32, kind="ExternalOutput").ap(),
    }
    with _TC(nc) as tc:
        with (
            tc.tile_pool(name="const", bufs=1) as p_const,
            tc.tile_pool(name="x", bufs=3) as p_x,
            tc.tile_pool(name="qt", bufs=4) as p_qt,
            tc.tile_pool(name="kt", bufs=1) as p_kt,
            tc.tile_pool(name="v2", bufs=1) as p_v2,
            tc.tile_pool(name="e", bufs=3) as p_e,
            tc.tile_pool(name="ot", bufs=8) as p_ot,
            tc.tile_pool(name="uo", bufs=4) as p_uo,
            tc.tile_pool(name="rl", bufs=2) as p_rl,
            tc.tile_pool(name="rl2", bufs=2) as p_rl2,
            tc.tile_pool(name="ob", bufs=3) as p_ob,
            tc.tile_pool(name="pa", bufs=2, space="PSUM") as p_pa,
            tc.tile_pool(name="sc", bufs=2, space="PSUM") as p_sc,
            tc.tile_pool(name="po", bufs=2, space="PSUM") as p_po,
        ):
            P = {
                "const": p_const,
                "x": p_x,
                "qt": p_qt,
                "kt": p_kt,
                "v2": p_v2,
                "e": p_e,
                "ot": p_ot,
                "uo": p_uo,
                "rl": p_rl,
                "rl2": p_rl2,
                "ob": p_ob,
                "pa": p_pa,
                "sc": p_sc,
                "po": p_po,
            }
            if hw_loop:
                with tc.For_i(0, hw_loop, 1):
                    _emit(nc, P, T)
            else:
                for _ in range(reps):
                    _emit(nc, P, T)
    return nc


def make_in_maps(x, Wq, bq, Wk, bk, Wv, bv, Wo, bo):
    """Host-side sharding: returns per-core input dicts."""
    scale = 1.0 / np.sqrt(np.float32(DH))
    xTs = [np.ascontiguousarray(x[b].T) for b in range(B)]
    in_maps = []
    for c in range(8):
        b = c // 4
        t = c % 4
        ch0 = t * CPC
        in_maps.append(
            {
                "xT": xTs[b],
                "wq": np.ascontiguousarray(Wq[:, ch0 : ch0 + CPC]) * scale,
                "wk": np.ascontiguousarray(Wk[:, ch0 : ch0 + CPC]),
                "wv": np.ascontiguousarray(Wv[:, ch0 : ch0 + CPC]),
                "wo": np.ascontiguousarray(Wo[ch0 : ch0 + CPC, :]),
                "bq": np.ascontiguousarray(
                    (bq[ch0 : ch0 + CPC] * scale).reshape(2, 128).T
                ),
                "bk": np.ascontiguousarray(bk[ch0 : ch0 + CPC].reshape(2, 128).T),
                "bvf": np.ascontiguousarray(np.broadcast_to(bv[ch0 : ch0 + CPC], (128, CPC))),
            }
        )
    return in_maps


def combine(results, bo):
    """Sum the 4 per-batch partials and add bo -> [B, S, D]."""
    out = np.zeros((B, S, D), np.float32)
    for c in range(8):
        out[c // 4] += results[c]["out"]
    return (out + bo.reshape(1, 1, D)).astype(np.float32)


def kernel(x, Wq, bq, Wk, bk, Wv, bv, Wo, bo):
    from concourse.bass_utils import run_bass_kernel_spmd

    args = [np.asarray(a, np.float32) for a in (x, Wq, bq, Wk, bk, Wv, bv, Wo, bo)]
    x, Wq, bq, Wk, bk, Wv, bv, Wo, bo = args
    nc = build(reps=1)
    in_maps = make_in_maps(x, Wq, bq, Wk, bk, Wv, bv, Wo, bo)
    res = run_bass_kernel_spmd(nc, in_maps, core_ids=list(range(8)))
    return combine(res.results, bo)


# revision 50
# speedup vs baseline: 1.0736x; 1.0187x over previous
"""Multi-head causal attention (B=2, S=2048, D=1024, H=16) on 8 trn2 cores.

Sharding: core c handles batch c//4 and heads 4*(c%4)..4*(c%4)+4 (256 channels).
Each core computes q/k/v projections for its channel slice, causal attention for
its 4 heads, and a partial output projection (contribution of its 256 channels
to the full [S, D] output). The host sums the 4 partials per batch and adds bo.

v2 schedule, built to keep the PE continuously busy (pstate ramp) and move all
softmax bookkeeping off the PE critical path:
  - qT/kT [256ch, 2048tok] via W-stationary matmuls over xT d-chunks; v in
    natural [tok, ch] layout with a ones column per head (v_aug) so the
    attention-value matmul also produces the softmax denominator; the ones
    columns are initialized by on-chip memset (no DRAM load).
  - scores^T for both heads of a 128-ch group land in one [128, 1024] PSUM
    tile (two banks); one exp covers both heads on off-diagonal k-tiles;
    diagonal k-tiles use per-head exp + DVE mask-multiply so the AV matmul
    never waits long. Masks are generated on-chip (gpsimd iota + compare).
  - scores for k-tile t+1 are emitted before the AV matmuls of k-tile t
    (depth-1 software pipeline) so the PE rarely waits on the exp chain.
  - normalization is fully off-PE: DVE reciprocal_approx_fast on the
    denominator row, gpsimd.partition_broadcast to spread it, DVE multiply.
  - input DMAs are issued in priority order (wk, xA-c0, wq, wv, ..., xA-c1)
    so the first projection chains start ~8us in; output stores ride the
    DVE queue, scratch copies the Sync queue.
"""

import sys

sys.path.insert(0, "/opt/trn_rl_repo")

import numpy as np
import concourse.bass as bass
import concourse.mybir as mybir
import concourse.tile as tile
from concourse.alu_op_type import AluOpType

F32R = mybir.dt.float32r
F32 = mybir.dt.float32
AF = mybir.ActivationFunctionType

D = 1024
S = 2048
B = 2
H = 16
DH = 64
CPC = 256  # channels per core (4 heads)
NKT = S // 128  # 16 k-tiles
V2W = 130  # [v0(64) | 1 | v1(64) | 1] per k-tile

_uid = [0]


def _split_waits(nc, max_waits=1):
    """This container's walrus rejects >max_waits sem-waits per instruction.
    Move excess waits onto preceding same-engine NoOps (one wait each);
    per-engine program order within a basic block preserves semantics."""
    n = 0
    for f in nc.m.functions:
        for b in f.blocks:
            insts = b.instructions
            if not any(
                i.sync_info is not None
                and i.sync_info.on_wait
                and len(i.sync_info.on_wait) > max_waits
                for i in insts
            ):
                continue
            new = []
            for inst in insts:
                si = inst.sync_info
                waits = list(si.on_wait) if si is not None and si.on_wait else []
                if len(waits) > max_waits:
                    for w in waits[max_waits:]:
                        _uid[0] += 1
                        new.append(
                            mybir.InstNoOp(
                                name=f"I-waitsplit-{_uid[0]}",
                                engine=inst.engine,
                                sync_info=mybir.SyncInfo(on_wait=[w], on_update=[]),
                            )
                        )
                        n += 1
                    si.on_wait = waits[:max_waits]
                new.append(inst)
            b.instructions = new
    return n


class _TC(tile.TileContext):
    def __exit__(self, exc_type, exc_val, exc_tb):
        r = super().__exit__(exc_type, exc_val, exc_tb)
        if exc_type is None:
            _split_waits(self.nc)
        return r


def _emit(nc, P, T):
    pc = P["const"]

    # --- constants / weights (mega tiles: [128 d-rows, 8 dc-chunks x cols]) ---
    wk_sb = pc.tile([128, 8 * CPC], F32R, tag="wk", name="wk")
    wq_sb = pc.tile([128, 8 * CPC], F32R, tag="wq", name="wq")
    wv_sb = pc.tile([128, 8 * CPC], F32R, tag="wv", name="wv")
    bq_sb = pc.tile([128, 2], F32, tag="bq", name="bq")
    bk_sb = pc.tile([128, 2], F32, tag="bk", name="bk")
    bvb = pc.tile([128, CPC], F32, tag="bvb", name="bvb")

    xa0 = P["x"].tile([128, 8 * 512], F32R, tag="x", name="xa0")
    xa1 = P["x"].tile([128, 8 * 512], F32R, tag="x", name="xa1")

    # priority-ordered input loads on the Sync queue
    nc.sync.dma_start(wk_sb[:].rearrange("p (a c) -> p a c", a=8),
                      T["wk"].rearrange("(a p) c -> p a c", p=128))
    nc.sync.dma_start(xa0[:].rearrange("p (a t) -> p a t", a=8),
                      T["xT"].rearrange("(a p) t -> p a t", p=128)[:, :, 0:512])
    nc.sync.dma_start(wq_sb[:].rearrange("p (a c) -> p a c", a=8),
                      T["wq"].rearrange("(a p) c -> p a c", p=128))
    nc.sync.dma_start(wv_sb[:].rearrange("p (a c) -> p a c", a=8),
                      T["wv"].rearrange("(a p) c -> p a c", p=128))
    nc.sync.dma_start(bq_sb[:], T["bq"])
    nc.sync.dma_start(bk_sb[:], T["bk"])
    nc.sync.dma_start(bvb[:], T["bvf"])
    nc.sync.dma_start(xa1[:].rearrange("p (a t) -> p a t", a=8),
                      T["xT"].rearrange("(a p) t -> p a t", p=128)[:, :, 512:1024])

    wparts = {"k": wk_sb, "q": wq_sb}
    xh = {(0, 0): xa0, (0, 1): xa1}  # (half, c) -> tile
    wo_box = [None]

    def xap(half, ci, dc, o0, w):
        """AP for d-chunk dc, cols [o0:o0+w] of the (half, ci) x block."""
        t = xh[(half, ci)]
        base = dc * 512
        return t[:, base + o0 : base + o0 + w]

    def wap(which, dc, g):
        t = wparts[which]
        base = dc * 256
        return t[:, base + g * 128 : base + g * 128 + 128]

    # causal masks [128 k, 2 heads x 512 q] per diagonal offset j, on-chip:
    # mask[j][k, :, q] = 1.0 if q - k - 128*j >= 0 else 0.0 (doubled so one
    # DVE multiply covers both heads of a merged exp tile)
    msk = []
    for j in range(4):
        m = pc.tile([128, 1024], F32R, tag=f"msk{j}", name=f"msk{j}")
        nc.gpsimd.iota(
            m[:],
            pattern=[[0, 2], [1, 512]],
            base=-128 * j,
            channel_multiplier=-1,
            allow_small_or_imprecise_dtypes=True,
        )
        nc.vector.tensor_scalar(m[:], m[:], 0.0, None, AluOpType.is_ge)
        msk.append(m)

    # f32r memset is broken in this walrus; build 1.0 columns via ACT
    # Identity with bias=1.0 from a zeroed f32 scratch (f32 memset works).
    zs = pc.tile([128, 64], F32, tag="zs", name="zs")
    nc.vector.memset(zs[:], 0.0)
    ones_sb = pc.tile([128, 64], F32R, tag="ones1", name="ones1")
    nc.scalar.activation(ones_sb[64:65, 0:64], zs[64:65, 0:64], AF.Identity, bias=1.0)



    # v_aug tiles; ones columns (64, 129) via the same ACT trick
    v2 = [
        [
            P["v2"].tile([128, V2W], F32R, tag=f"v2_{g}_{tt}", name=f"v2_{g}_{tt}")
            for tt in range(NKT)
        ]
        for g in range(2)
    ]
    for g in range(2):
        for tt in range(NKT):
            nc.scalar.activation(
                v2[g][tt][:].rearrange("p (a c) -> p a c", a=2)[:, :, 64:65],
                zs[:, 0:2],
                AF.Identity,
                bias=1.0,
            )

    qt = {}  # (g, c) -> [128ch, 512tok]
    kt = {
        (g, c): P["kt"].tile([128, 512], F32R, tag=f"kt{g}_{c}", name=f"kt{g}_{c}")
        for g in range(2)
        for c in range(4)
    }
    ot = {}  # (g, qc) -> [128ch, 512tok] normalized attention output

    def qk_chain(which, g, c):
        """One q or k projection chain: 8 accumulating matmuls + bias-evac."""
        half, ci = divmod(c, 2)
        bsb = bk_sb if which == "k" else bq_sb
        ps = P["pa"].tile([128, 512], F32, tag="pa", name="pa")
        for dc in range(8):
            yield "pe", lambda ps=ps, dc=dc, which=which, half=half, ci=ci, g=g: nc.tensor.matmul(
                ps[:],
                wap(which, dc, g),
                xap(half, ci, dc, 0, 512),
                start=(dc == 0),
                stop=(dc == 7),
            )

        def _evac(ps=ps, which=which, g=g, c=c, bsb=bsb):
            if which == "k":
                dst = kt[(g, c)]
            else:
                dst = P["qt"].tile([128, 512], F32R, tag="qt", name=f"qt{g}_{c}")
                qt[(g, c)] = dst
            nc.vector.tensor_scalar_add(dst[:], ps[:], bsb[:, g : g + 1])

        yield "x", _evac

    def v_chain(tt):
        """One v projection chain (128 tokens, all 256 ch): 8 matmuls + 2 evacs."""
        half, tl = divmod(tt, 8)
        ci, t128 = divmod(tl, 4)
        ps = P["pa"].tile([128, 512], F32, tag="pa", name="pa")
        for dc in range(8):
            yield "pe", lambda ps=ps, dc=dc, half=half, ci=ci, t128=t128: nc.tensor.matmul(
                ps[:, 0:256],
                xap(half, ci, dc, t128 * 128, 128),
                wv_sb[:, dc * 256 : (dc + 1) * 256],
                start=(dc == 0),
                stop=(dc == 7),
            )
        for g in range(2):
            yield "x", lambda ps=ps, g=g, tt=tt: nc.vector.tensor_add(
                v2[g][tt][:].rearrange("p (a c) -> p a c", a=2)[:, :, 0:64],
                ps[:, g * 128 : (g + 1) * 128].rearrange("p (a c) -> p a c", a=2),
                bvb[:, g * 128 : (g + 1) * 128].rearrange("p (a c) -> p a c", a=2),
            )

    def outproj(qc):
        """Partial out-projection for 512 tokens: 16 matmuls + evac/store."""
        for tl in range(4):
            tt = qc * 4 + tl
            for nch in range(2):
                ps = P["pa"].tile([128, 512], F32, tag="pa", name="pa")
                for g in range(2):
                    yield "pe", lambda ps=ps, g=g, qc=qc, tl=tl, nch=nch: nc.tensor.matmul(
                        ps[:],
                        ot[(g, qc)][:, tl * 128 : (tl + 1) * 128],
                        wo_box[0][:, g * D + nch * 512 : g * D + nch * 512 + 512],
                        start=(g == 0),
                        stop=(g == 1),
                    )

                def _evac(ps=ps, tt=tt, nch=nch):
                    ob = P["ob"].tile([128, 512], F32, tag="ob", name="ob")
                    # alternate evac engine so neither DVE nor ACT serializes
                    # the pa-pool rotation in the drain tail
                    if nch == 0:
                        nc.vector.tensor_scalar_add(ob[:], ps[:], 0.0)
                    else:
                        nc.scalar.copy(ob[:], ps[:])
                    nc.sync.dma_start(
                        T["out"][tt * 128 : (tt + 1) * 128, nch * 512 : (nch + 1) * 512],
                        ob[:],
                    )

                yield "x", _evac

    # --- filler machinery: inject deferred PE work between attention matmuls.
    # Units are ("pe", fn) or ("x", fn); fill(n) emits until n PE units ran so
    # evac/recip units don't eat the PE's fill budget. ---
    fill_q = []
    norm_gens = set()

    def fill(n_pe, cap=8):
        while n_pe > 0 and cap > 0 and fill_q:
            u = next(fill_q[0], None)
            if u is None:
                norm_gens.discard(fill_q[0])
                fill_q.pop(0)
                continue
            kind, fn = u
            fn()
            cap -= 1
            if kind == "pe":
                n_pe -= 1

    def drain_front():
        gen = fill_q.pop(0)
        norm_gens.discard(gen)
        for _, fn in gen:
            fn()

    def drain_until(marker):
        """Drain everything queued ahead of (and including) marker, but keep
        norm generators paced via fill slots — bursting them would park the
        PE behind the ACT reciprocal chain."""
        stash = []
        while any(g is marker for g in fill_q):
            if fill_q[0] in norm_gens:
                stash.append(fill_q.pop(0))
                continue
            drain_front()
        fill_q[0:0] = stash

    def attention_seg(qc, g):
        """Causal attention for one 128-ch head pair over 512 queries.

        AV matmuls trail the score matmuls by two k-tiles so the exp+mask
        chain (ACT+DVE) never gates the PE; sc double-buffering gives the
        same slack to the score matmuls."""
        Oh = [None, None]
        nkt = 4 * qc + 4
        es = {}

        def av_pair(kti):
            if Oh[0] is None:
                Oh[0] = P["po"].tile([128, 512], F32, tag="po", name="po")
                Oh[1] = P["po"].tile([128, 512], F32, tag="po", name="po")
            e, off = es.pop(kti)
            for par in range(2):
                nc.tensor.matmul(
                    Oh[par][0:65, off:512],
                    v2[g][kti][:, par * 65 : par * 65 + 65],
                    e[:, par * 512 + off : (par + 1) * 512],
                    start=(kti == 0),
                    stop=(kti == nkt - 1),
                )

        for kti in range(nkt):
            ktile = kt[(g, kti // 4)]
            k0 = (kti % 4) * 128
            j = kti - 4 * qc
            # diag tiles: only cols [off:] are unmasked; keep N>=256 for f32r
            off = 0 if j < 1 else min(128 * j, 256)
            sc = P["sc"].tile([128, 1024], F32, tag="sc", name="sc")
            for par in range(2):
                # explicit row-tiles: the two K=64 score matmuls run
                # concurrently in disjoint halves of the PE array
                nc.tensor.matmul(
                    sc[:, par * 512 + off : (par + 1) * 512],
                    ktile[64 * par : 64 * par + 64, k0 : k0 + 128],
                    qt[(g, qc)][64 * par : 64 * par + 64, off:512],
                    start=True,
                    stop=True,
                    tile_position=(64 * par, 0),
                )
            e = P["e"].tile([128, 1024], F32R, tag="e", name="e")
            ev = e[:].rearrange("p (a q) -> p a q", a=2)[:, :, off:512]
            scv = sc[:].rearrange("p (a q) -> p a q", a=2)[:, :, off:512]
            nc.scalar.activation(ev, scv, AF.Exp)
            if j >= 0:
                mv = msk[j][:].rearrange("p (a q) -> p a q", a=2)[:, :, off:512]
                nc.vector.tensor_mul(ev, ev, mv)
            es[kti] = (e, off)
            fill(2)
            if kti >= 3:
                av_pair(kti - 3)
        av_pair(nkt - 3)
        fill(1)
        av_pair(nkt - 2)
        fill(1)
        av_pair(nkt - 1)

        # evacuate the unnormalized head outputs (+ denominator row) to SBUF
        # right away so the two PSUM banks recycle for the next segment
        uo = []
        for par in range(2):
            u = P["uo"].tile([128, 512], F32R, tag="uo", name="uo")
            nc.vector.tensor_scalar_add(u[0:65, :], Oh[par][0:65, :], 0.0)
            uo.append(u)

        # normalization, fully deferred through the fill queue so the PE and
        # ACT streams never stall on it: ACT rl2 = exp(-ln(l)) = 1/l on the
        # denominator row, then PE ones-matmul broadcast, DVE evac + multiply.
        dst = P["ot"].tile([128, 512], F32R, tag="ot", name=f"ot{g}_{qc}")
        ot[(g, qc)] = dst

        def norm_units(uo=uo, dst=dst):
            rl2s = []

            def _recip(par):
                rl = P["rl"].tile([128, 512], F32R, tag="rl", name="rl")
                with nc.allow_low_precision(reason="softmax denominators, ~1e-3"):
                    nc.scalar.activation(rl[64:65, :], uo[par][64:65, :], AF.Ln)
                    rl2 = P["rl2"].tile([128, 512], F32R, tag="rl2", name="rl2")
                    nc.scalar.activation(
                        rl2[64:65, :], rl[64:65, :], AF.Exp, scale=-1.0
                    )
                rl2s.append(rl2)

            yield "x", lambda: _recip(0)
            yield "x", lambda: _recip(1)
            for par in range(2):
                rlb = P["pa"].tile([128, 512], F32, tag="pa", name="rlb")
                yield "pe", lambda rlb=rlb, par=par: nc.tensor.matmul(
                    rlb[0:64, :],
                    ones_sb[64:65, 0:64],
                    rl2s[par][64:65, :],
                    start=True,
                    stop=True,
                )

                def _mul(rlb=rlb, par=par):
                    rlb_sb = P["rl"].tile([128, 512], F32R, tag="rlbsb", name="rlbsb")
                    nc.vector.tensor_scalar_add(rlb_sb[0:64, :], rlb[0:64, :], 0.0)
                    if par == 0:
                        nc.vector.tensor_mul(
                            dst[0:64, :], uo[par][0:64, :], rlb_sb[0:64, :]
                        )
                    else:
                        tmp = P["rl"].tile([128, 512], F32R, tag="otmp", name="otmp")
                        nc.vector.tensor_mul(
                            tmp[0:64, :], uo[par][0:64, :], rlb_sb[0:64, :]
                        )
                        nc.sync.dma_start(dst[64:128, :], tmp[0:64, :])

                yield "x", _mul

        gen = norm_units()
        norm_gens.add(gen)
        fill_q.insert(0, gen)

    # ---- schedule ----
    # inline: the chains the first attention segment needs
    for which, g in (("k", 0), ("k", 1), ("q", 0), ("q", 1)):
        for _, fn in qk_chain(which, g, 0):
            fn()
    for tt in range(4):
        for _, fn in v_chain(tt):
            fn()

    # half-B x loads + wo: xb0 reuses xa0's pool buffer, whose readers (the
    # c0 chains above) are all emitted; triggers queue behind the startup
    # loads on the Sync FIFO, preserving bandwidth priority.
    xb0 = P["x"].tile([128, 8 * 512], F32R, tag="x", name="xb0")
    xb1 = P["x"].tile([128, 8 * 512], F32R, tag="x", name="xb1")
    nc.sync.dma_start(xb0[:].rearrange("p (a t) -> p a t", a=8),
                      T["xT"].rearrange("(a p) t -> p a t", p=128)[:, :, 1024:1536])
    nc.sync.dma_start(xb1[:].rearrange("p (a t) -> p a t", a=8),
                      T["xT"].rearrange("(a p) t -> p a t", p=128)[:, :, 1536:2048])
    wo = pc.tile([128, 2 * D], F32R, tag="wo", name="wo")
    nc.sync.dma_start(wo[:].rearrange("p (t n) -> p t n", t=2),
                      T["wo"].rearrange("(t p) n -> p t n", p=128))
    xh[(1, 0)] = xb0
    xh[(1, 1)] = xb1
    wo_box[0] = wo

    # deferred work, in the order later segments need it
    for which, g in (("k", 0), ("k", 1), ("q", 0), ("q", 1)):
        fill_q.append(qk_chain(which, g, 1))
    pre_qc = {}
    for tt in range(4, 8):
        gen = v_chain(tt)
        fill_q.append(gen)
    pre_qc[1] = gen
    for which, g in (("k", 0), ("k", 1), ("q", 0), ("q", 1)):
        fill_q.append(qk_chain(which, g, 2))
    for tt in range(8, 12):
        gen = v_chain(tt)
        fill_q.append(gen)
    pre_qc[2] = gen
    for which, g in (("k", 0), ("k", 1), ("q", 0), ("q", 1)):
        fill_q.append(qk_chain(which, g, 3))
    for tt in range(12, 16):
        gen = v_chain(tt)
        fill_q.append(gen)
    pre_qc[3] = gen

    for qc in range(4):
        if qc in pre_qc:
            drain_until(pre_qc[qc])
        for g in range(2):
            attention_seg(qc, g)
        fill_q.append(outproj(qc))
    # final drain: give the last norm's ACT reciprocal time to land by
    # emitting one other generator's PE work ahead of it. The last element is
    # always outproj(3), which needs the norm emitted first — only swap when
    # fill_q[1] is an earlier (already-satisfied) generator.
    if fill_q and fill_q[0] in norm_gens and len(fill_q) > 2:
        fill_q[0], fill_q[1] = fill_q[1], fill_q[0]
    while fill_q:
        drain_front()


def build(reps=1, with_bias=True, hw_loop=0):
    nc = bass.Bass("TRN2", target_bir_lowering=False, debug=False, num_devices=8)
    T = {
        "xT": nc.dram_tensor("xT", [D, S], F32R, kind="ExternalInput").ap(),
        "wq": nc.dram_tensor("wq", [D, CPC], F32R, kind="ExternalInput").ap(),
        "wk": nc.dram_tensor("wk", [D, CPC], F32R, kind="ExternalInput").ap(),
        "wv": nc.dram_tensor("wv", [D, CPC], F32R, kind="ExternalInput").ap(),
        "wo": nc.dram_tensor("wo", [CPC, D], F32R, kind="ExternalInput").ap(),
        "bq": nc.dram_tensor("bq", [128, 2], F32, kind="ExternalInput").ap(),
        "bk": nc.dram_tensor("bk", [128, 2], F32, kind="ExternalInput").ap(),
        "bvf": nc.dram_tensor("bvf", [128, CPC], F32, kind="ExternalInput").ap(),
        "out": nc.dram_tensor("out", [S, D], F32, kind="ExternalOutput").ap(),
    }
    with _TC(nc) as tc:
        with (
            tc.tile_pool(name="const", bufs=1) as p_const,
            tc.tile_pool(name="x", bufs=3) as p_x,
            tc.tile_pool(name="qt", bufs=4) as p_qt,
            tc.tile_pool(name="kt", bufs=1) as p_kt,
            tc.tile_pool(name="v2", bufs=1) as p_v2,
            tc.tile_pool(name="e", bufs=4) as p_e,
            tc.tile_pool(name="ot", bufs=8) as p_ot,
            tc.tile_pool(name="uo", bufs=4) as p_uo,
            tc.tile_pool(name="rl", bufs=2) as p_rl,
            tc.tile_pool(name="rl2", bufs=2) as p_rl2,
            tc.tile_pool(name="ob", bufs=3) as p_ob,
            tc.tile_pool(name="pa", bufs=2, space="PSUM") as p_pa,
            tc.tile_pool(name="sc", bufs=2, space="PSUM") as p_sc,
            tc.tile_pool(name="po", bufs=2, space="PSUM") as p_po,
        ):
            P = {
                "const": p_const,
                "x": p_x,
                "qt": p_qt,
                "kt": p_kt,
                "v2": p_v2,
                "e": p_e,
                "ot": p_ot,
                "uo": p_uo,
                "rl": p_rl,
                "rl2": p_rl2,
                "ob": p_ob,
                "pa": p_pa,
                "sc": p_sc,
                "po": p_po,
            }
            if hw_loop:
                with tc.For_i(0, hw_loop, 1):
                    _emit(nc, P, T)
            else:
                for _ in range(reps):
                    _emit(nc, P, T)
    return nc


def make_in_maps(x, Wq, bq, Wk, bk, Wv, bv, Wo, bo):
    """Host-side sharding: returns per-core input dicts."""
    scale = 1.0 / np.sqrt(np.float32(DH))
    xTs = [np.ascontiguousarray(x[b].T) for b in range(B)]
    in_maps = []
    for c in range(8):
        b = c // 4
        t = c % 4
        ch0 = t * CPC
        in_maps.append(
            {
                "xT": xTs[b],
                "wq": np.ascontiguousarray(Wq[:, ch0 : ch0 + CPC]) * scale,
                "wk": np.ascontiguousarray(Wk[:, ch0 : ch0 + CPC]),
                "wv": np.ascontiguousarray(Wv[:, ch0 : ch0 + CPC]),
                "wo": np.ascontiguousarray(Wo[ch0 : ch0 + CPC, :]),
                "bq": np.ascontiguousarray(
                    (bq[ch0 : ch0 + CPC] * scale).reshape(2, 128).T
                ),
                "bk": np.ascontiguousarray(bk[ch0 : ch0 + CPC].reshape(2, 128).T),
                "bvf": np.ascontiguousarray(np.broadcast_to(bv[ch0 : ch0 + CPC], (128, CPC))),
            }
        )
    return in_maps


def combine(results, bo):
    """Sum the 4 per-batch partials and add bo -> [B, S, D]."""
    out = np.zeros((B, S, D), np.float32)
    for c in range(8):
        out[c // 4] += results[c]["out"]
    return (out + bo.reshape(1, 1, D)).astype(np.float32)


def kernel(x, Wq, bq, Wk, bk, Wv, bv, Wo, bo):
    from concourse.bass_utils import run_bass_kernel_spmd

    args = [np.asarray(a, np.float32) for a in (x, Wq, bq, Wk, bk, Wv, bv, Wo, bo)]
    x, Wq, bq, Wk, bk, Wv, bv, Wo, bo = args
    nc = build(reps=1)
    in_maps = make_in_maps(x, Wq, bq, Wk, bk, Wv, bv, Wo, bo)
    res = run_bass_kernel_spmd(nc, in_maps, core_ids=list(range(8)))
    return combine(res.results, bo)


# revision 56
# speedup vs baseline: 1.0757x; 1.0020x over previous
"""Multi-head causal attention (B=2, S=2048, D=1024, H=16) on 8 trn2 cores.

Sharding: core c handles batch c//4 and heads 4*(c%4)..4*(c%4)+4 (256 channels).
Each core computes q/k/v projections for its channel slice, causal attention for
its 4 heads, and a partial output projection (contribution of its 256 channels
to the full [S, D] output). The host sums the 4 partials per batch and adds bo.

v2 schedule, built to keep the PE continuously busy (pstate ramp) and move all
softmax bookkeeping off the PE critical path:
  - qT/kT [256ch, 2048tok] via W-stationary matmuls over xT d-chunks; v in
    natural [tok, ch] layout with a ones column per head (v_aug) so the
    attention-value matmul also produces the softmax denominator; the ones
    columns are initialized by on-chip memset (no DRAM load).
  - scores^T for both heads of a 128-ch group land in one [128, 1024] PSUM
    tile (two banks); one exp covers both heads on off-diagonal k-tiles;
    diagonal k-tiles use per-head exp + DVE mask-multiply so the AV matmul
    never waits long. Masks are generated on-chip (gpsimd iota + compare).
  - scores for k-tile t+1 are emitted before the AV matmuls of k-tile t
    (depth-1 software pipeline) so the PE rarely waits on the exp chain.
  - normalization is fully off-PE: DVE reciprocal_approx_fast on the
    denominator row, gpsimd.partition_broadcast to spread it, DVE multiply.
  - input DMAs are issued in priority order (wk, xA-c0, wq, wv, ..., xA-c1)
    so the first projection chains start ~8us in; output stores ride the
    DVE queue, scratch copies the Sync queue.
"""

import sys

sys.path.insert(0, "/opt/trn_rl_repo")

import numpy as np
import concourse.bass as bass
import concourse.mybir as mybir
import concourse.tile as tile
from concourse.alu_op_type import AluOpType

F32R = mybir.dt.float32r
F32 = mybir.dt.float32
AF = mybir.ActivationFunctionType

D = 1024
S = 2048
B = 2
H = 16
DH = 64
CPC = 256  # channels per core (4 heads)
NKT = S // 128  # 16 k-tiles
V2W = 130  # [v0(64) | 1 | v1(64) | 1] per k-tile

_uid = [0]


def _split_waits(nc, max_waits=1):
    """This container's walrus rejects >max_waits sem-waits per instruction.
    Move excess waits onto preceding same-engine NoOps (one wait each);
    per-engine program order within a basic block preserves semantics."""
    n = 0
    for f in nc.m.functions:
        for b in f.blocks:
            insts = b.instructions
            if not any(
                i.sync_info is not None
                and i.sync_info.on_wait
                and len(i.sync_info.on_wait) > max_waits
                for i in insts
            ):
                continue
            new = []
            for inst in insts:
                si = inst.sync_info
                waits = list(si.on_wait) if si is not None and si.on_wait else []
                if len(waits) > max_waits:
                    for w in waits[max_waits:]:
                        _uid[0] += 1
                        new.append(
                            mybir.InstNoOp(
                                name=f"I-waitsplit-{_uid[0]}",
                                engine=inst.engine,
                                sync_info=mybir.SyncInfo(on_wait=[w], on_update=[]),
                            )
                        )
                        n += 1
                    si.on_wait = waits[:max_waits]
                new.append(inst)
            b.instructions = new
    return n


class _TC(tile.TileContext):
    def __exit__(self, exc_type, exc_val, exc_tb):
        r = super().__exit__(exc_type, exc_val, exc_tb)
        if exc_type is None:
            _split_waits(self.nc)
        return r


def _emit(nc, P, T):
    pc = P["const"]

    # --- constants / weights, split lo/hi (d-chunks 0-3 / 4-7) so the first
    # projection chains can start after ~1.5MB of input instead of 3MB ---
    wk_p = [pc.tile([128, 4 * CPC], F32R, tag=f"wk{h}", name=f"wk{h}") for h in range(2)]
    wq_p = [pc.tile([128, 4 * CPC], F32R, tag=f"wq{h}", name=f"wq{h}") for h in range(2)]
    wv_sb = pc.tile([128, 8 * CPC], F32R, tag="wv", name="wv")
    bq_sb = pc.tile([128, 2], F32, tag="bq", name="bq")
    bk_sb = pc.tile([128, 2], F32, tag="bk", name="bk")
    bvb = pc.tile([128, CPC], F32, tag="bvb", name="bvb")

    xparts = {}

    def xalloc(half, ci, name):
        xparts[(half, ci)] = [
            P["x"].tile([128, 4 * 512], F32R, tag="x", name=f"{name}{h}")
            for h in range(2)
        ]

    def xload(half, ci, h):
        t0 = 1024 * half + 512 * ci
        nc.sync.dma_start(
            xparts[(half, ci)][h][:].rearrange("p (a t) -> p a t", a=4),
            T["xT"].rearrange("(a p) t -> p a t", p=128)[:, 4 * h : 4 * h + 4, t0 : t0 + 512],
        )

    xalloc(0, 0, "xa0")
    xalloc(0, 1, "xa1")

    # priority-ordered input loads on the Sync queue
    wdram = {"k": T["wk"], "q": T["wq"]}
    def wload(which, parts, h):
        nc.sync.dma_start(
            parts[h][:].rearrange("p (a c) -> p a c", a=4),
            wdram[which].rearrange("(a p) c -> p a c", p=128)[:, 4 * h : 4 * h + 4],
        )

    wload("k", wk_p, 0)
    xload(0, 0, 0)
    wload("k", wk_p, 1)
    xload(0, 0, 1)
    wload("q", wq_p, 0)
    wload("q", wq_p, 1)
    nc.sync.dma_start(wv_sb[:].rearrange("p (a c) -> p a c", a=8),
                      T["wv"].rearrange("(a p) c -> p a c", p=128))
    nc.sync.dma_start(bq_sb[:], T["bq"])
    nc.sync.dma_start(bk_sb[:], T["bk"])
    nc.sync.dma_start(bvb[:], T["bvf"])
    xload(0, 1, 0)
    xload(0, 1, 1)

    wparts = {"k": wk_p, "q": wq_p}
    wo_box = [None]

    def xap(half, ci, dc, o0, w):
        """AP for d-chunk dc, cols [o0:o0+w] of the (half, ci) x block."""
        t = xparts[(half, ci)][dc // 4]
        base = (dc % 4) * 512
        return t[:, base + o0 : base + o0 + w]

    def wap(which, dc, g):
        t = wparts[which][dc // 4]
        base = (dc % 4) * 256
        return t[:, base + g * 128 : base + g * 128 + 128]

    # causal masks [128 k, 2 heads x 512 q] per diagonal offset j, on-chip:
    # mask[j][k, :, q] = 1.0 if q - k - 128*j >= 0 else 0.0 (doubled so one
    # DVE multiply covers both heads of a merged exp tile)
    msk = []
    for j in range(4):
        m = pc.tile([128, 1024], F32R, tag=f"msk{j}", name=f"msk{j}")
        nc.gpsimd.iota(
            m[:],
            pattern=[[0, 2], [1, 512]],
            base=-128 * j,
            channel_multiplier=-1,
            allow_small_or_imprecise_dtypes=True,
        )
        nc.vector.tensor_scalar(m[:], m[:], 0.0, None, AluOpType.is_ge)
        msk.append(m)

    # f32r memset is broken in this walrus; build 1.0 columns via ACT
    # Identity with bias=1.0 from a zeroed f32 scratch (f32 memset works).
    zs = pc.tile([128, 64], F32, tag="zs", name="zs")
    nc.vector.memset(zs[:], 0.0)
    ones_sb = pc.tile([128, 64], F32R, tag="ones1", name="ones1")
    nc.scalar.activation(ones_sb[64:65, 0:64], zs[64:65, 0:64], AF.Identity, bias=1.0)



    # v_aug tiles; ones columns (64, 129) via the same ACT trick
    v2 = [
        [
            P["v2"].tile([128, V2W], F32R, tag=f"v2_{g}_{tt}", name=f"v2_{g}_{tt}")
            for tt in range(NKT)
        ]
        for g in range(2)
    ]
    for g in range(2):
        for tt in range(NKT):
            nc.scalar.activation(
                v2[g][tt][:].rearrange("p (a c) -> p a c", a=2)[:, :, 64:65],
                zs[:, 0:2],
                AF.Identity,
                bias=1.0,
            )

    qt = {}  # (g, c) -> [128ch, 512tok]
    kt = {
        (g, c): P["kt"].tile([128, 512], F32R, tag=f"kt{g}_{c}", name=f"kt{g}_{c}")
        for g in range(2)
        for c in range(4)
    }
    ot = {}  # (g, qc) -> [128ch, 512tok] normalized attention output

    def qk_chain(which, g, c):
        """One q or k projection chain: 8 accumulating matmuls + bias-evac."""
        half, ci = divmod(c, 2)
        bsb = bk_sb if which == "k" else bq_sb
        ps = P["pa"].tile([128, 512], F32, tag="pa", name="pa")
        for dc in range(8):
            yield "pe", lambda ps=ps, dc=dc, which=which, half=half, ci=ci, g=g: nc.tensor.matmul(
                ps[:],
                wap(which, dc, g),
                xap(half, ci, dc, 0, 512),
                start=(dc == 0),
                stop=(dc == 7),
            )

        def _evac(ps=ps, which=which, g=g, c=c, bsb=bsb):
            if which == "k":
                dst = kt[(g, c)]
            else:
                dst = P["qt"].tile([128, 512], F32R, tag="qt", name=f"qt{g}_{c}")
                qt[(g, c)] = dst
            nc.vector.tensor_scalar_add(dst[:], ps[:], bsb[:, g : g + 1])

        yield "x", _evac

    def v_chain(tt):
        """One v projection chain (128 tokens, all 256 ch): 8 matmuls + 2 evacs."""
        half, tl = divmod(tt, 8)
        ci, t128 = divmod(tl, 4)
        ps = P["pa"].tile([128, 512], F32, tag="pa", name="pa")
        for dc in range(8):
            yield "pe", lambda ps=ps, dc=dc, half=half, ci=ci, t128=t128: nc.tensor.matmul(
                ps[:, 0:256],
                xap(half, ci, dc, t128 * 128, 128),
                wv_sb[:, dc * 256 : (dc + 1) * 256],
                start=(dc == 0),
                stop=(dc == 7),
            )
        for g in range(2):
            yield "x", lambda ps=ps, g=g, tt=tt: nc.vector.tensor_add(
                v2[g][tt][:].rearrange("p (a c) -> p a c", a=2)[:, :, 0:64],
                ps[:, g * 128 : (g + 1) * 128].rearrange("p (a c) -> p a c", a=2),
                bvb[:, g * 128 : (g + 1) * 128].rearrange("p (a c) -> p a c", a=2),
            )

    def outproj(qc):
        """Partial out-projection for 512 tokens: 16 matmuls + evac/store."""
        for tl in range(4):
            tt = qc * 4 + tl
            for nch in range(2):
                ps = P["pa"].tile([128, 512], F32, tag="pa", name="pa")
                for g in range(2):
                    yield "pe", lambda ps=ps, g=g, qc=qc, tl=tl, nch=nch: nc.tensor.matmul(
                        ps[:],
                        ot[(g, qc)][:, tl * 128 : (tl + 1) * 128],
                        wo_box[0][:, g * D + nch * 512 : g * D + nch * 512 + 512],
                        start=(g == 0),
                        stop=(g == 1),
                    )

                def _evac(ps=ps, tt=tt, nch=nch):
                    ob = P["ob"].tile([128, 512], F32, tag="ob", name="ob")
                    # alternate evac engine so neither DVE nor ACT serializes
                    # the pa-pool rotation in the drain tail
                    if nch == 0:
                        nc.vector.tensor_scalar_add(ob[:], ps[:], 0.0)
                    else:
                        nc.scalar.copy(ob[:], ps[:])
                    nc.sync.dma_start(
                        T["out"][tt * 128 : (tt + 1) * 128, nch * 512 : (nch + 1) * 512],
                        ob[:],
                    )

                yield "x", _evac

    # --- filler machinery: inject deferred PE work between attention matmuls.
    # Units are ("pe", fn) or ("x", fn); fill(n) emits until n PE units ran so
    # evac/recip units don't eat the PE's fill budget. ---
    fill_q = []
    norm_gens = set()

    def fill(n_pe, cap=8):
        while n_pe > 0 and cap > 0 and fill_q:
            u = next(fill_q[0], None)
            if u is None:
                norm_gens.discard(fill_q[0])
                fill_q.pop(0)
                continue
            kind, fn = u
            fn()
            cap -= 1
            if kind == "pe":
                n_pe -= 1

    def drain_front():
        gen = fill_q.pop(0)
        norm_gens.discard(gen)
        for _, fn in gen:
            fn()

    def drain_until(marker):
        """Drain everything queued ahead of (and including) marker, but keep
        norm generators paced via fill slots — bursting them would park the
        PE behind the ACT reciprocal chain."""
        stash = []
        while any(g is marker for g in fill_q):
            if fill_q[0] in norm_gens:
                stash.append(fill_q.pop(0))
                continue
            drain_front()
        for i, g in enumerate(stash):
            fill_q.insert(min(2 * i, len(fill_q)), g)

    def attention_seg(qc, g):
        """Causal attention for one 128-ch head pair over 512 queries.

        AV matmuls trail the score matmuls by two k-tiles so the exp+mask
        chain (ACT+DVE) never gates the PE; sc double-buffering gives the
        same slack to the score matmuls."""
        Oh = [None, None]
        nkt = 4 * qc + 4
        es = {}

        def av_pair(kti):
            if Oh[0] is None:
                Oh[0] = P["po"].tile([128, 512], F32, tag="po", name="po")
                Oh[1] = P["po"].tile([128, 512], F32, tag="po", name="po")
            e, off = es.pop(kti)
            for par in range(2):
                nc.tensor.matmul(
                    Oh[par][0:65, off:512],
                    v2[g][kti][:, par * 65 : par * 65 + 65],
                    e[:, par * 512 + off : (par + 1) * 512],
                    start=(kti == 0),
                    stop=(kti == nkt - 1),
                )

        for kti in range(nkt):
            ktile = kt[(g, kti // 4)]
            k0 = (kti % 4) * 128
            j = kti - 4 * qc
            # diag tiles: only cols [off:] are unmasked; keep N>=256 for f32r
            off = 0 if j < 1 else min(128 * j, 256)
            sc = P["sc"].tile([128, 1024], F32, tag="sc", name="sc")
            for par in range(2):
                # explicit row-tiles: the two K=64 score matmuls run
                # concurrently in disjoint halves of the PE array
                nc.tensor.matmul(
                    sc[:, par * 512 + off : (par + 1) * 512],
                    ktile[64 * par : 64 * par + 64, k0 : k0 + 128],
                    qt[(g, qc)][64 * par : 64 * par + 64, off:512],
                    start=True,
                    stop=True,
                    tile_position=(64 * par, 0),
                )
            e = P["e"].tile([128, 1024], F32R, tag="e", name="e")
            ev = e[:].rearrange("p (a q) -> p a q", a=2)[:, :, off:512]
            scv = sc[:].rearrange("p (a q) -> p a q", a=2)[:, :, off:512]
            nc.scalar.activation(ev, scv, AF.Exp)
            if j >= 0:
                mv = msk[j][:].rearrange("p (a q) -> p a q", a=2)[:, :, off:512]
                nc.vector.tensor_mul(ev, ev, mv)
            es[kti] = (e, off)
            fill(2)
            if kti >= 3:
                av_pair(kti - 3)
        av_pair(nkt - 3)
        fill(1)
        av_pair(nkt - 2)
        fill(1)
        av_pair(nkt - 1)

        # evacuate the unnormalized head outputs (+ denominator row) to SBUF
        # right away so the two PSUM banks recycle for the next segment
        uo = []
        for par in range(2):
            u = P["uo"].tile([128, 512], F32R, tag="uo", name="uo")
            nc.vector.tensor_scalar_add(u[0:65, :], Oh[par][0:65, :], 0.0)
            uo.append(u)

        # normalization, fully deferred through the fill queue so the PE and
        # ACT streams never stall on it: ACT rl2 = exp(-ln(l)) = 1/l on the
        # denominator row, then PE ones-matmul broadcast, DVE evac + multiply.
        dst = P["ot"].tile([128, 512], F32R, tag="ot", name=f"ot{g}_{qc}")
        ot[(g, qc)] = dst

        rl2s = []

        def norm_recips(uo=uo):
            def _recip(par):
                rl = P["rl"].tile([128, 512], F32R, tag="rl", name="rl")
                with nc.allow_low_precision(reason="softmax denominators, ~1e-3"):
                    nc.scalar.activation(rl[64:65, :], uo[par][64:65, :], AF.Ln)
                    rl2 = P["rl2"].tile([128, 512], F32R, tag="rl2", name="rl2")
                    nc.scalar.activation(
                        rl2[64:65, :], rl[64:65, :], AF.Exp, scale=-1.0
                    )
                rl2s.append(rl2)

            yield "x", lambda: _recip(0)
            yield "x", lambda: _recip(1)

        def norm_apply(uo=uo, dst=dst, rl2s=rl2s):
            for par in range(2):
                rlb = P["pa"].tile([128, 512], F32, tag="pa", name="rlb")
                yield "pe", lambda rlb=rlb, par=par: nc.tensor.matmul(
                    rlb[0:64, :],
                    ones_sb[64:65, 0:64],
                    rl2s[par][64:65, :],
                    start=True,
                    stop=True,
                )

                def _mul(rlb=rlb, par=par):
                    rlb_sb = P["rl"].tile([128, 512], F32R, tag="rlbsb", name="rlbsb")
                    nc.vector.tensor_scalar_add(rlb_sb[0:64, :], rlb[0:64, :], 0.0)
                    if par == 0:
                        nc.vector.tensor_mul(
                            dst[0:64, :], uo[par][0:64, :], rlb_sb[0:64, :]
                        )
                    else:
                        tmp = P["rl"].tile([128, 512], F32R, tag="otmp", name="otmp")
                        nc.vector.tensor_mul(
                            tmp[0:64, :], uo[par][0:64, :], rlb_sb[0:64, :]
                        )
                        nc.sync.dma_start(dst[64:128, :], tmp[0:64, :])

                yield "x", _mul

        # recips go to the queue front; the dependent PE broadcast goes one
        # generator later so the fill pacing gives the ACT chain time to land
        genA = norm_recips()
        genB = norm_apply()
        norm_gens.add(genA)
        norm_gens.add(genB)
        fill_q.insert(0, genA)
        fill_q.insert(min(2, len(fill_q)), genB)

    # ---- schedule ----
    # inline: the chains the first attention segment needs
    for which, g in (("k", 0), ("k", 1), ("q", 0), ("q", 1)):
        for _, fn in qk_chain(which, g, 0):
            fn()
    for tt in range(4):
        for _, fn in v_chain(tt):
            fn()

    # half-B x loads + wo: xb1 reuses xa0's pool buffers, whose readers (the
    # c0 chains above) are all emitted; triggers queue behind the startup
    # loads on the Sync FIFO, preserving bandwidth priority.
    xalloc(1, 0, "xb0")
    xalloc(1, 1, "xb1")
    for h in range(2):
        xload(1, 0, h)
    for h in range(2):
        xload(1, 1, h)
    wo = pc.tile([128, 2 * D], F32R, tag="wo", name="wo")
    nc.sync.dma_start(wo[:].rearrange("p (t n) -> p t n", t=2),
                      T["wo"].rearrange("(t p) n -> p t n", p=128))
    wo_box[0] = wo

    # deferred work, in the order later segments need it
    for which, g in (("k", 0), ("k", 1), ("q", 0), ("q", 1)):
        fill_q.append(qk_chain(which, g, 1))
    pre_qc = {}
    for tt in range(4, 8):
        gen = v_chain(tt)
        fill_q.append(gen)
    pre_qc[1] = gen
    for which, g in (("k", 0), ("k", 1), ("q", 0), ("q", 1)):
        fill_q.append(qk_chain(which, g, 2))
    for tt in range(8, 12):
        gen = v_chain(tt)
        fill_q.append(gen)
    pre_qc[2] = gen
    for which, g in (("k", 0), ("k", 1), ("q", 0), ("q", 1)):
        fill_q.append(qk_chain(which, g, 3))
    for tt in range(12, 16):
        gen = v_chain(tt)
        fill_q.append(gen)
    pre_qc[3] = gen

    for qc in range(4):
        if qc in pre_qc:
            drain_until(pre_qc[qc])
        for g in range(2):
            attention_seg(qc, g)
        fill_q.append(outproj(qc))
    # final drain: give the last norm's ACT reciprocal time to land by
    # emitting one other generator's PE work ahead of it. The last element is
    # always outproj(3), which needs the norm emitted first — only swap when
    # fill_q[1] is an earlier (already-satisfied) generator.
    if (
        fill_q
        and fill_q[0] in norm_gens
        and len(fill_q) > 2
        and fill_q[1] not in norm_gens
    ):
        fill_q[0], fill_q[1] = fill_q[1], fill_q[0]
    while fill_q:
        drain_front()


def build(reps=1, with_bias=True, hw_loop=0):
    nc = bass.Bass("TRN2", target_bir_lowering=False, debug=False, num_devices=8)
    T = {
        "xT": nc.dram_tensor("xT", [D, S], F32R, kind="ExternalInput").ap(),
        "wq": nc.dram_tensor("wq", [D, CPC], F32R, kind="ExternalInput").ap(),
        "wk": nc.dram_tensor("wk", [D, CPC], F32R, kind="ExternalInput").ap(),
        "wv": nc.dram_tensor("wv", [D, CPC], F32R, kind="ExternalInput").ap(),
        "wo": nc.dram_tensor("wo", [CPC, D], F32R, kind="ExternalInput").ap(),
        "bq": nc.dram_tensor("bq", [128, 2], F32, kind="ExternalInput").ap(),
        "bk": nc.dram_tensor("bk", [128, 2], F32, kind="ExternalInput").ap(),
        "bvf": nc.dram_tensor("bvf", [128, CPC], F32, kind="ExternalInput").ap(),
        "out": nc.dram_tensor("out", [S, D], F32, kind="ExternalOutput").ap(),
    }
    with _TC(nc) as tc:
        with (
            tc.tile_pool(name="const", bufs=1) as p_const,
            tc.tile_pool(name="x", bufs=6) as p_x,
            tc.tile_pool(name="qt", bufs=4) as p_qt,
            tc.tile_pool(name="kt", bufs=1) as p_kt,
            tc.tile_pool(name="v2", bufs=1) as p_v2,
            tc.tile_pool(name="e", bufs=4) as p_e,
            tc.tile_pool(name="ot", bufs=8) as p_ot,
            tc.tile_pool(name="uo", bufs=4) as p_uo,
            tc.tile_pool(name="rl", bufs=2) as p_rl,
            tc.tile_pool(name="rl2", bufs=2) as p_rl2,
            tc.tile_pool(name="ob", bufs=3) as p_ob,
            tc.tile_pool(name="pa", bufs=2, space="PSUM") as p_pa,
            tc.tile_pool(name="sc", bufs=2, space="PSUM") as p_sc,
            tc.tile_pool(name="po", bufs=2, space="PSUM") as p_po,
        ):
            P = {
                "const": p_const,
                "x": p_x,
                "qt": p_qt,
                "kt": p_kt,
                "v2": p_v2,
                "e": p_e,
                "ot": p_ot,
                "uo": p_uo,
                "rl": p_rl,
                "rl2": p_rl2,
                "ob": p_ob,
                "pa": p_pa,
                "sc": p_sc,
                "po": p_po,
            }
            if hw_loop:
                with tc.For_i(0, hw_loop, 1):
                    _emit(nc, P, T)
            else:
                for _ in range(reps):
                    _emit(nc, P, T)
    return nc


def make_in_maps(x, Wq, bq, Wk, bk, Wv, bv, Wo, bo):
    """Host-side sharding: returns per-core input dicts."""
    scale = 1.0 / np.sqrt(np.float32(DH))
    xTs = [np.ascontiguousarray(x[b].T) for b in range(B)]
    in_maps = []
    for c in range(8):
        b = c // 4
        t = c % 4
        ch0 = t * CPC
        in_maps.append(
            {
                "xT": xTs[b],
                "wq": np.ascontiguousarray(Wq[:, ch0 : ch0 + CPC]) * scale,
                "wk": np.ascontiguousarray(Wk[:, ch0 : ch0 + CPC]),
                "wv": np.ascontiguousarray(Wv[:, ch0 : ch0 + CPC]),
                "wo": np.ascontiguousarray(Wo[ch0 : ch0 + CPC, :]),
                "bq": np.ascontiguousarray(
                    (bq[ch0 : ch0 + CPC] * scale).reshape(2, 128).T
                ),
                "bk": np.ascontiguousarray(bk[ch0 : ch0 + CPC].reshape(2, 128).T),
                "bvf": np.ascontiguousarray(np.broadcast_to(bv[ch0 : ch0 + CPC], (128, CPC))),
            }
        )
    return in_maps


def combine(results, bo):
    """Sum the 4 per-batch partials and add bo -> [B, S, D]."""
    out = np.zeros((B, S, D), np.float32)
    for c in range(8):
        out[c // 4] += results[c]["out"]
    return (out + bo.reshape(1, 1, D)).astype(np.float32)


def kernel(x, Wq, bq, Wk, bk, Wv, bv, Wo, bo):
    from concourse.bass_utils import run_bass_kernel_spmd

    args = [np.asarray(a, np.float32) for a in (x, Wq, bq, Wk, bk, Wv, bv, Wo, bo)]
    x, Wq, bq, Wk, bk, Wv, bv, Wo, bo = args
    nc = build(reps=1)
    in_maps = make_in_maps(x, Wq, bq, Wk, bk, Wv, bv, Wo, bo)
    res = run_bass_kernel_spmd(nc, in_maps, core_ids=list(range(8)))
    return combine(res.results, bo)
